# revision 1
# baseline (speedup 1.0000x reference)
"""Trainium2 Bass kernel for BasicTransformerBlockST (spatial/temporal transformer block).

Sharding over 8 NeuronCores:
  Phase A (spatial self-attn): data-parallel over (b,t): core i owns the 4
  groups bt = i + 8g, so every core holds both batches.
  An 8-way on-device AllToAll reshards to (b,h,w)-parallel: core j owns rows
  (b=j//4, hw in [144*(j%4), 144*(j%4+1))), tokens r-major (token = r*16 + t).
  Phases temporal-1, cross-attn, temporal-2, FFN run on that shard.

Matmul operands bf16 (fp32 PSUM accumulation); residual stream, LN and softmax
statistics fp32. Residual stream lives in DRAM between phases.
"""

import sys

sys.path.insert(0, "/opt/trn_rl_repo")

import numpy as np
import ml_dtypes

import concourse.bass as bass
import concourse.bacc as bacc
import concourse.mybir as mybir
import concourse.tile as tile
from concourse.masks import make_identity

F32 = mybir.dt.float32
BF16 = mybir.dt.bfloat16
AF = mybir.ActivationFunctionType
ALU = mybir.AluOpType

B, C, T, H, W = 2, 640, 16, 24, 24
HEADS, DH = 8, 80
CTXD = 1024
MAXREL = 16
NREL = 2 * MAXREL + 1          # 33
FFI = 4 * C                    # 2560
INNER = HEADS * DH             # 640
SCALE = DH ** -0.5
EPS = 1e-5

NCORES = 8
NG = 4                         # spatial groups per core
SEQ = H * W                    # 576
NR = (B * H * W) // NCORES     # 144 rows per core
TOK = NR * T                   # 2304 tokens per core
NWIN = TOK // 128              # 18
GW = 5                         # windows per padded spatial group
CHUNKS = C // 128              # 5
CTXCH = CTXD // 128            # 8
HALFW = NWIN // 2              # 9 windows per temporal half
HR = NR // 2                   # 72 rows per half


def nsplits(n, cap=512):
    out, o = [], 0
    while o < n:
        out.append((o, min(cap, n - o)))
        o += min(cap, n - o)
    return out


def build_program(debug=False):
    nc = bacc.Bacc(None, target_bir_lowering=False)

    xs_in = nc.dram_tensor("xs_in", [NG, SEQ, C], F32, kind="ExternalInput")
    ctxT_in = nc.dram_tensor("ctxT", [CTXD, 77], BF16, kind="ExternalInput")

    def win(name, shape, dt=BF16):
        return nc.dram_tensor(name, shape, dt, kind="ExternalInput")

    wts, biases = {}, {}
    for p in ("a1", "a2", "t1", "t2"):
        cin = CTXD if p == "a2" else C
        wts[f"{p}_wq"] = win(f"{p}_wq", [C, INNER])
        wts[f"{p}_wk"] = win(f"{p}_wk", [cin, INNER])
        wts[f"{p}_wv"] = win(f"{p}_wv", [cin, INNER])
        wts[f"{p}_wo"] = win(f"{p}_wo", [DH, HEADS, C])
        biases[f"{p}_bq"] = win(f"{p}_bq", [INNER], F32)
        biases[f"{p}_bk"] = win(f"{p}_bk", [INNER], F32)
        biases[f"{p}_bv"] = win(f"{p}_bv", [INNER], F32)
        biases[f"{p}_bo"] = win(f"{p}_bo", [C], F32)
    for p in ("t1", "t2"):
        wts[f"{p}_rkT"] = win(f"{p}_rkT", [DH, NREL])
        wts[f"{p}_rvs"] = win(f"{p}_rvs", [16, T, DH])  # rvs[j,t,d]=rv[j-t+16,d]
    wts["ff_w1"] = win("ff_w1", [C, 2 * FFI])
    wts["ff_w2"] = win("ff_w2", [FFI, C])
    biases["ff_b1"] = win("ff_b1", [2 * FFI], F32)
    biases["ff_b2"] = win("ff_b2", [C], F32)
    bd_mask = win("bd_mask", [128, 128], F32)

    out_final = nc.dram_tensor("out", [NR, T, C], F32, kind="ExternalOutput")
    dbg = {}
    if debug:
        dbg["a"] = nc.dram_tensor("dbg_a", [NG, SEQ, C], F32, kind="ExternalOutput")
        for nm in ("t1", "x2", "t2"):
            dbg[nm] = nc.dram_tensor(f"dbg_{nm}", [NR, T, C], F32,
                                     kind="ExternalOutput")

    a2a_in = nc.dram_tensor("a2a_in", [NCORES, NR, 2, C], F32)
    a2a_out = nc.dram_tensor("a2a_out", [NCORES, NR, 2, C], F32)
    x_dram = nc.dram_tensor("x_dram", [TOK, C], F32)
    sim2_dram = nc.dram_tensor("sim2_dram", [TOK, HEADS, 16], BF16)
    groups = [[0, 1, 2, 3, 4, 5, 6, 7]]

    from contextlib import ExitStack

    with tile.TileContext(nc) as tc, ExitStack() as top:
        const = top.enter_context(tc.tile_pool(name="const", bufs=1))
        ident = const.tile([128, 128], F32)
        make_identity(nc, ident)
        identb = const.tile([128, 128], BF16)
        make_identity(nc, identb)
        eps_t = const.tile([128, 1], F32)
        nc.vector.memset(eps_t[:], EPS)
        mask = const.tile([128, 128], F32)
        nc.sync.dma_start(out=mask[:], in_=bd_mask[:, :])
        small = top.enter_context(tc.tile_pool(name="small", bufs=4))
        zscr = top.enter_context(tc.tile_pool(name="zscr", bufs=2))

        def bcast_tile(wp, name, n=C):
            t = wp.tile([128, n], F32, tag=f"bc_{name}")
            src = biases[name][:]
            bc = bass.AP(tensor=src.tensor, offset=src.offset,
                         ap=[[0, 128], [1, n]])
            nc.gpsimd.dma_start(out=t[:], in_=bc)
            return t

        # ---------------- shared helpers ----------------
        def ln_to_fm(psp, x_ap, zT_tile, nw):
            """LN over channels + transpose: x [128,nw,640] f32 ->
            zT [128,CHUNKS,nw*128] bf16 feature-major (normalized, no g/b)."""
            for w in range(nw):
                x = x_ap[:, w, :]
                st = small.tile([128, CHUNKS, 6], F32, tag="bnst")
                for s in range(CHUNKS):
                    nc.vector.bn_stats(out=st[:, s, :],
                                       in_=x[:, 128 * s:128 * (s + 1)])
                mv = small.tile([128, 2], F32, tag="bnmv")
                nc.vector.bn_aggr(out=mv[:], in_=st[:])
                rstd = small.tile([128, 1], F32, tag="rstd")
                nc.scalar.activation(out=rstd[:], in_=mv[:, 1:2], func=AF.Sqrt,
                                     bias=eps_t[:], scale=1.0)
                nc.vector.reciprocal(out=rstd[:], in_=rstd[:])
                zs = zscr.tile([128, C], F32, tag="zs")
                nc.vector.tensor_scalar(
                    out=zs[:], in0=x, scalar1=mv[:, 0:1], scalar2=rstd[:],
                    op0=ALU.subtract, op1=ALU.mult)
                for c in range(CHUNKS):
                    pt = psp.tile([128, 128], F32, tag="ps")
                    nc.tensor.transpose(pt[:], zs[:, 128 * c:128 * (c + 1)],
                                        ident[:])
                    nc.vector.tensor_scalar_mul(
                        out=zT_tile[:, c, 128 * w:128 * (w + 1)], in0=pt[:],
                        scalar1=1.0)

        def proj_heads(psp, zT, w_sb, out_tile, ntok, bias=None,
                       cin_chunks=CHUNKS):
            """per-head feature-major projection: out [80, HEADS, ntok] bf16."""
            for h in range(HEADS):
                for (o, n) in nsplits(ntok):
                    pt = psp.tile([128, 512], F32, tag="ps")
                    for ci in range(cin_chunks):
                        nc.tensor.matmul(pt[:DH, :n],
                                         w_sb[:, ci, DH * h:DH * (h + 1)],
                                         zT[:, ci, o:o + n],
                                         start=(ci == 0),
                                         stop=(ci == cin_chunks - 1))
                    if bias is not None:
                        nc.vector.tensor_scalar_add(out=out_tile[:, h, o:o + n],
                                                    in0=pt[:DH, :n],
                                                    scalar1=bias[:, h:h + 1])
                    else:
                        nc.vector.tensor_scalar_mul(out=out_tile[:, h, o:o + n],
                                                    in0=pt[:DH, :n], scalar1=1.0)

        def proj_tm(psp, zT, w_sb, out_tile, tok_chunks, badd=None):
            """token-major: out[tok, 640]; lhsT = zT[:,ci,toks], rhs = W."""
            for (w, p, toff) in tok_chunks:
                for (o, n) in nsplits(C):
                    pt = psp.tile([128, 512], F32, tag="ps")
                    for ci in range(CHUNKS):
                        nc.tensor.matmul(pt[:p, :n],
                                         zT[:, ci, toff:toff + p],
                                         w_sb[:, ci, o:o + n],
                                         start=(ci == 0), stop=(ci == CHUNKS - 1))
                    if badd is not None:
                        nc.vector.tensor_add(out=out_tile[:p, w, o:o + n],
                                             in0=pt[:p, :n],
                                             in1=badd[:p, o:o + n])
                    else:
                        nc.scalar.copy(out=out_tile[:p, w, o:o + n],
                                       in_=pt[:p, :n])

        def wo_residual(psp, oT, wo, w, resid_ap, bo):
            """by-head wo projection + bias + residual-add into resid_ap."""
            mp = resid_ap.shape[0]
            for (o, n) in nsplits(C):
                pw = psp.tile([128, 512], F32, tag="ps")
                for h in range(HEADS):
                    nc.tensor.matmul(pw[:mp, :n],
                                     oT[:, h, 128 * w:128 * w + mp],
                                     wo[:, h, o:o + n],
                                     start=(h == 0), stop=(h == HEADS - 1))
                nc.vector.tensor_add(out=resid_ap[:, o:o + n], in0=pw[:mp, :n],
                                     in1=resid_ap[:, o:o + n])
            nc.vector.tensor_add(out=resid_ap[:], in0=resid_ap[:], in1=bo[:mp, :])

        def load_w_cin(wp, name, cin):
            t = wp.tile([128, cin // 128, wts[name].shape[-1]], BF16, tag=name[3:])
            nc.sync.dma_start(out=t[:],
                              in_=wts[name][:].rearrange("(a p) n -> p a n", p=128))
            return t

        def load_wo(wp, name):
            t = wp.tile([DH, HEADS, C], BF16, tag="wo")
            nc.sync.dma_start(out=t[:], in_=wts[name][:])
            return t

        def load_bias_h(wp, name):
            t = wp.tile([DH, HEADS], F32, tag=name[3:] + "b")
            nc.sync.dma_start(out=t[:],
                              in_=biases[name][:].rearrange("(h p) -> p h", p=DH))
            return t

        # =====================================================================
        # PHASE A: spatial self-attention, per (b,t) group
        # =====================================================================
        with ExitStack() as ph:
            wp = ph.enter_context(tc.tile_pool(name="wpA", bufs=1))
            zp = ph.enter_context(tc.tile_pool(name="zpA", bufs=1))
            qp = ph.enter_context(tc.tile_pool(name="qpA", bufs=2))
            ap_ = ph.enter_context(tc.tile_pool(name="apA", bufs=1))
            psp = ph.enter_context(tc.tile_pool(name="psA", bufs=8, space="PSUM"))

            wq = load_w_cin(wp, "a1_wq", C)
            wk = load_w_cin(wp, "a1_wk", C)
            wv = load_w_cin(wp, "a1_wv", C)
            wo = load_wo(wp, "a1_wo")
            bq = load_bias_h(wp, "a1_bq")
            bk = load_bias_h(wp, "a1_bk")
            bv_b = bcast_tile(wp, "a1_bv")
            bo_b = bcast_tile(wp, "a1_bo")

            tok_chunks = [(w, 128 if w < 4 else 64, 128 * w) for w in range(GW)]

            for g in range(NG):
                xg = zp.tile([128, GW, C], F32, tag="xa")
                nc.sync.dma_start(out=xg[:, 0:4, :],
                                  in_=xs_in[g, 0:512, :].rearrange(
                                      "(a p) c -> p a c", p=128))
                nc.sync.dma_start(out=xg[:64, 4, :], in_=xs_in[g, 512:576, :])
                nc.vector.memset(xg[64:128, 4, :], 0.0)

                zT = zp.tile([128, CHUNKS, GW * 128], BF16, tag="zTa")
                ln_to_fm(psp, xg, zT, GW)

                qT = qp.tile([DH, HEADS, SEQ], BF16, tag="qa")
                kT = qp.tile([DH, HEADS, SEQ], BF16, tag="ka")
                proj_heads(psp, zT[:, :, 0:SEQ], wq, qT, SEQ, bias=bq)
                proj_heads(psp, zT[:, :, 0:SEQ], wk, kT, SEQ, bias=bk)
                v = qp.tile([128, GW, C], BF16, tag="va")
                proj_tm(psp, zT, wv, v, tok_chunks, badd=bv_b)

                oT = ap_.tile([DH, HEADS, SEQ], BF16, tag="oa")
                for h in range(HEADS):
                    a_sb = ap_.tile([128, GW, SEQ], BF16, tag="aa")
                    for (mw, mp, moff) in tok_chunks:
                        zsum = small.tile([128, 2], F32, tag="zs2")
                        ex = ap_.tile([128, SEQ], F32, tag="ex")
                        for ki, (o, n) in enumerate(nsplits(SEQ)):
                            ps = psp.tile([128, 512], F32, tag="ps")
                            nc.tensor.matmul(ps[:mp, :n],
                                             qT[:, h, moff:moff + mp],
                                             kT[:, h, o:o + n],
                                             start=True, stop=True)
                            nc.scalar.activation(
                                out=ex[:mp, o:o + n], in_=ps[:mp, :n],
                                func=AF.Exp, scale=SCALE,
                                accum_out=zsum[:mp, ki:ki + 1])
                        ztot = small.tile([128, 1], F32, tag="zt")
                        nc.vector.tensor_add(out=ztot[:mp, :], in0=zsum[:mp, 0:1],
                                             in1=zsum[:mp, 1:2])
                        nc.vector.reciprocal(out=ztot[:mp, :], in_=ztot[:mp, :])
                        nc.vector.tensor_scalar_mul(out=a_sb[:mp, mw, :],
                                                    in0=ex[:mp, :],
                                                    scalar1=ztot[:mp, :])
                    # AV: o^T[d, q] = sum_k v[k, d] a[q, k]; query chunks
                    # paired so each AV matmul streams N=256.
                    pairs = [(tok_chunks[0], tok_chunks[1]),
                             (tok_chunks[2], tok_chunks[3]),
                             (tok_chunks[4], None)]
                    for (c0, c1) in pairs:
                        np_ = c0[1] + (c1[1] if c1 else 0)
                        moff = c0[2]
                        po = psp.tile([DH, 256], F32, tag="ps")
                        for ik, (kw, kp, koff) in enumerate(tok_chunks):
                            aT = ap_.tile([128, 256], BF16, tag="aT")
                            for sub, cc in enumerate((c0, c1)):
                                if cc is None:
                                    continue
                                (mw, mp, mo) = cc
                                pa = psp.tile([128, 128], BF16, tag="ps")
                                nc.tensor.transpose(pa[:kp, :mp],
                                                    a_sb[:mp, mw, koff:koff + kp],
                                                    identb[:mp, :mp])
                                nc.scalar.copy(out=aT[:kp, 128 * sub:128 * sub + mp],
                                               in_=pa[:kp, :mp])
                            nc.tensor.matmul(po[:, :np_],
                                             v[:kp, kw, DH * h:DH * (h + 1)],
                                             aT[:kp, :np_] if np_ == 256 else
                                             aT[:kp, :np_],
                                             start=(ik == 0), stop=(ik == GW - 1))
                        nc.scalar.copy(out=oT[:, h, moff:moff + np_],
                                       in_=po[:, :np_])

                for (mw, mp, moff) in tok_chunks:
                    xn = zp.tile([128, C], F32, tag="xan")
                    nc.scalar.copy(out=xn[:mp, :], in_=xg[:mp, mw, :])
                    wo_residual(psp, oT, wo, mw, xn[:mp, :], bo_b)
                    q0, q1 = moff // NR, (moff + mp - 1) // NR
                    for q in range(q0, q1 + 1):
                        lo, hi = max(moff, NR * q), min(moff + mp, NR * (q + 1))
                        nc.sync.dma_start(
                            out=a2a_in[4 * (g // 2) + q, lo - NR * q:hi - NR * q,
                                       g % 2, :],
                            in_=xn[lo - moff:hi - moff, :])
                    if debug:
                        nc.sync.dma_start(out=dbg["a"][g, moff:moff + mp, :],
                                          in_=xn[:mp, :])

        # =====================================================================
        # AllToAll reshard
        # =====================================================================
        nc.gpsimd.collective_compute("AllToAll", ALU.bypass, replica_groups=groups,
                                     ins=[a2a_in[:]], outs=[a2a_out[:]])

        def load_x_window(dst_ap, wg, first):
            if first:
                base = a2a_out[:]
                src = bass.AP(tensor=base.tensor,
                              offset=base.offset + 8 * wg * 2 * C,
                              ap=[[2 * C, 8], [C, 2], [NR * 2 * C, 8], [1, C]])
            else:
                src = x_dram[128 * wg:128 * (wg + 1), :]
            nc.sync.dma_start(out=dst_ap, in_=src)

        # =====================================================================
        # Temporal attention (t1 / t2)
        # =====================================================================
        def temporal(prefix, dbg_key, first):
            with ExitStack() as ph:
                wp = ph.enter_context(tc.tile_pool(name="wpT", bufs=1))
                zp = ph.enter_context(tc.tile_pool(name="zpT", bufs=1))
                qp = ph.enter_context(tc.tile_pool(name="qpT", bufs=1))
                ap_ = ph.enter_context(tc.tile_pool(name="apT", bufs=2))
                op_ = ph.enter_context(tc.tile_pool(name="opT", bufs=1))
                psp = ph.enter_context(tc.tile_pool(name="psT", bufs=8,
                                                    space="PSUM"))

                wq = load_w_cin(wp, f"{prefix}_wq", C)
                wk = load_w_cin(wp, f"{prefix}_wk", C)
                wv = load_w_cin(wp, f"{prefix}_wv", C)
                wo = load_wo(wp, f"{prefix}_wo")
                bq = load_bias_h(wp, f"{prefix}_bq")
                bk = load_bias_h(wp, f"{prefix}_bk")
                bv_b = bcast_tile(wp, f"{prefix}_bv")
                bo_b = bcast_tile(wp, f"{prefix}_bo")
                rkT = wp.tile([DH, NREL], BF16, tag="rkT")
                nc.sync.dma_start(out=rkT[:], in_=wts[f"{prefix}_rkT"][:])
                rvs = wp.tile([16, T, DH], BF16, tag="rvs")
                nc.sync.dma_start(out=rvs[:], in_=wts[f"{prefix}_rvs"][:])

                for half in range(2):
                    wlo = half * HALFW
                    ntok = 128 * HALFW  # 1152
                    xw = zp.tile([128, HALFW, C], F32, tag="xw")
                    for w in range(HALFW):
                        load_x_window(xw[:, w, :], wlo + w, first)
                    zT = zp.tile([128, CHUNKS, ntok], BF16, tag="zTt")
                    ln_to_fm(psp, xw, zT, HALFW)

                    qT = qp.tile([DH, HEADS, ntok], BF16, tag="qt")
                    kT = qp.tile([DH, HEADS, ntok], BF16, tag="kt")
                    proj_heads(psp, zT, wq, qT, ntok, bias=bq)
                    proj_heads(psp, zT, wk, kT, ntok, bias=bk)
                    v = qp.tile([128, HALFW, C], BF16, tag="vt")
                    proj_tm(psp, zT, wv, v,
                            [(w, 128, 128 * w) for w in range(HALFW)],
                            badd=bv_b)

                    # rel-pos scores P^T = rk . q^T; shear-transpose into
                    # sim2 token layout, bounce via DRAM.
                    s2byT = ap_.tile([HR, T, HEADS, 16], BF16, tag="s2byT")
                    for h in range(HEADS):
                        pSB = ap_.tile([NREL, ntok], BF16, tag="pSB")
                        for (o, n) in nsplits(ntok):
                            pp = psp.tile([NREL, 512], F32, tag="ps")
                            nc.tensor.matmul(pp[:, :n], rkT[:, :],
                                             qT[:, h, o:o + n],
                                             start=True, stop=True)
                            nc.scalar.copy(out=pSB[:, o:o + n], in_=pp[:, :n])
                        for t in range(T):
                            src = bass.AP(tensor=pSB.tensor,
                                          offset=pSB[:, :].offset + t,
                                          ap=[list(pSB[:, :].ap[0]), [16, HR]])
                            pt = psp.tile([HR, NREL], BF16, tag="ps")
                            nc.tensor.transpose(pt[:], src, identb[:NREL, :NREL])
                            nc.scalar.copy(
                                out=s2byT[:, t, h, :],
                                in_=pt[:, MAXREL - t:2 * MAXREL - t])
                    dst = sim2_dram[:].rearrange("(r t) h j -> r t h j", t=T)
                    nc.sync.dma_start(out=dst[HR * half:HR * half + HR],
                                      in_=s2byT[:])

                    # attention windows
                    oT = op_.tile([DH, HEADS, ntok], BF16, tag="ot")
                    aDT = op_.tile([16, HEADS, ntok], BF16, tag="aDT")
                    for w in range(HALFW):
                        wg = wlo + w
                        s2w = ap_.tile([128, HEADS, 16], BF16, tag="s2w")
                        nc.sync.dma_start(
                            out=s2w[:],
                            in_=sim2_dram[128 * wg:128 * (wg + 1), :, :])
                        aG = ap_.tile([128, HEADS, 128], BF16, tag="aG")
                        for h in range(HEADS):
                            ps = psp.tile([128, 128], F32, tag="ps")
                            nc.tensor.matmul(ps[:],
                                             qT[:, h, 128 * w:128 * (w + 1)],
                                             kT[:, h, 128 * w:128 * (w + 1)],
                                             start=True, stop=True)
                            s2rep = bass.AP(
                                tensor=s2w.tensor,
                                offset=s2w[:, h, :].offset,
                                ap=[list(s2w[:, :, :].ap[0]), [0, 8], [1, 16]])
                            tmp = ap_.tile([128, 128], F32, tag="tmpst")
                            nc.vector.scalar_tensor_tensor(
                                out=tmp[:], in0=ps[:], scalar=1.0, in1=s2rep,
                                op0=ALU.mult, op1=ALU.add)
                            exv = ap_.tile([128, 128], F32, tag="exv")
                            nc.scalar.activation(out=exv[:], in_=tmp[:],
                                                 func=AF.Exp, scale=SCALE)
                            zsum = small.tile([128, 1], F32, tag="zsT")
                            nc.vector.scalar_tensor_tensor(
                                out=aG[:, h, :], in0=exv[:], scalar=1.0,
                                in1=mask[:], op0=ALU.mult, op1=ALU.mult,
                                accum_out=zsum[:])
                            nc.vector.reciprocal(out=zsum[:], in_=zsum[:])
                            nc.vector.tensor_scalar_mul(out=aG[:, h, :],
                                                        in0=aG[:, h, :],
                                                        scalar1=zsum[:])
                        # within-row diag blocks: off-diag of aG is zero, so
                        # aD[p,h,j] = sum_g' aG[p,h,16g'+j]
                        aD = ap_.tile([128, HEADS, 16], F32, tag="aD")
                        agv = bass.AP(
                            tensor=aG.tensor, offset=aG[:, :, :].offset,
                            ap=[list(aG[:, :, :].ap[0]), [128, HEADS],
                                [1, 16], [16, 8]])
                        nc.vector.tensor_reduce(
                            out=aD[:], in_=agv, axis=mybir.AxisListType.X,
                            op=ALU.add)
                        for h in range(HEADS):
                            paT = psp.tile([128, 128], BF16, tag="ps")
                            nc.tensor.transpose(paT[:], aG[:, h, :], identb[:])
                            aTs = ap_.tile([128, 128], BF16, tag="aTs")
                            nc.scalar.copy(out=aTs[:], in_=paT[:])
                            po = psp.tile([DH, 128], F32, tag="ps")
                            nc.tensor.matmul(po[:], v[:, w, DH * h:DH * (h + 1)],
                                             aTs[:], start=True, stop=True)
                            nc.scalar.copy(out=oT[:, h, 128 * w:128 * (w + 1)],
                                           in_=po[:])
                            pd = psp.tile([16, 128], F32, tag="ps")
                            nc.tensor.transpose(pd[:], aD[:, h, :], ident[:])
                            nc.scalar.copy(out=aDT[:, h, 128 * w:128 * (w + 1)],
                                           in_=pd[:])
                    # o2: per (t, h): lhsT = pre-shifted rv slice, rhs = aDT cols
                    for t in range(T):
                        for h in range(HEADS):
                            rhs = bass.AP(tensor=aDT.tensor,
                                          offset=aDT[:, h, :].offset + t,
                                          ap=[list(aDT[:, :, :].ap[0]), [16, HR]])
                            p2 = psp.tile([DH, HR], F32, tag="ps")
                            nc.tensor.matmul(p2[:], rvs[:, t, :], rhs,
                                             start=True, stop=True)
                            dstp = bass.AP(tensor=oT.tensor,
                                           offset=oT[:, h, :].offset + t,
                                           ap=[list(oT[:, :, :].ap[0]), [16, HR]])
                            nc.vector.tensor_add(out=dstp, in0=p2[:], in1=dstp)
                    # wo + residual, store to x_dram
                    for w in range(HALFW):
                        wg = wlo + w
                        wo_residual(psp, oT, wo, w, xw[:, w, :], bo_b)
                        nc.sync.dma_start(out=x_dram[128 * wg:128 * (wg + 1), :],
                                          in_=xw[:, w, :])
                        if debug:
                            nc.sync.dma_start(
                                out=dbg[dbg_key][:].rearrange(
                                    "r t c -> (r t) c")[128 * wg:128 * (wg + 1), :],
                                in_=xw[:, w, :])

        temporal("t1", "t1", first=True)

        # =====================================================================
        # Cross-attention (processed in halves)
        # =====================================================================
        with ExitStack() as ph:
            wp = ph.enter_context(tc.tile_pool(name="wpX", bufs=1))
            zp = ph.enter_context(tc.tile_pool(name="zpX", bufs=2))
            qp = ph.enter_context(tc.tile_pool(name="qpX", bufs=1))
            ap_ = ph.enter_context(tc.tile_pool(name="apX", bufs=2))
            op_ = ph.enter_context(tc.tile_pool(name="opX", bufs=2))
            psp = ph.enter_context(tc.tile_pool(name="psX", bufs=8, space="PSUM"))

            wq = load_w_cin(wp, "a2_wq", C)
            wkc = load_w_cin(wp, "a2_wk", CTXD)
            wvc = load_w_cin(wp, "a2_wv", CTXD)
            wo = load_wo(wp, "a2_wo")
            bq = load_bias_h(wp, "a2_bq")
            bo_b = bcast_tile(wp, "a2_bo")

            ctx_sb = wp.tile([128, CTXCH, 77], BF16, tag="ctx")
            nc.sync.dma_start(out=ctx_sb[:],
                              in_=ctxT_in[:].rearrange("(a p) m -> p a m", p=128))
            kctxT = wp.tile([DH, HEADS, 77], BF16, tag="kctx")
            for h in range(HEADS):
                pt = psp.tile([128, 77], F32, tag="ps")
                for ci in range(CTXCH):
                    nc.tensor.matmul(pt[:DH, :], wkc[:, ci, DH * h:DH * (h + 1)],
                                     ctx_sb[:, ci, :],
                                     start=(ci == 0), stop=(ci == CTXCH - 1))
                nc.scalar.copy(out=kctxT[:, h, :], in_=pt[:DH, :])
            vctx = wp.tile([77, C], BF16, tag="vctx")
            for (o, n) in nsplits(C):
                pt = psp.tile([77, 512], F32, tag="ps")
                for ci in range(CTXCH):
                    nc.tensor.matmul(pt[:, :n], ctx_sb[:, ci, :],
                                     wvc[:, ci, o:o + n],
                                     start=(ci == 0), stop=(ci == CTXCH - 1))
                nc.scalar.copy(out=vctx[:, o:o + n], in_=pt[:, :n])

            for half in range(2):
                wlo = half * HALFW
                ntok = 128 * HALFW
                xw = zp.tile([128, HALFW, C], F32, tag="xwx")
                for w in range(HALFW):
                    load_x_window(xw[:, w, :], wlo + w, False)
                zT = zp.tile([128, CHUNKS, ntok], BF16, tag="zTx")
                ln_to_fm(psp, xw, zT, HALFW)
                qT = qp.tile([DH, HEADS, ntok], BF16, tag="qx")
                proj_heads(psp, zT, wq, qT, ntok, bias=bq)

                oT = op_.tile([DH, HEADS, ntok], BF16, tag="ox")
                for w in range(HALFW):
                    for h in range(HEADS):
                        ps = psp.tile([128, 77], F32, tag="ps")
                        nc.tensor.matmul(ps[:], qT[:, h, 128 * w:128 * (w + 1)],
                                         kctxT[:, h, :], start=True, stop=True)
                        ex = ap_.tile([128, 77], F32, tag="exx")
                        zsum = small.tile([128, 1], F32, tag="zsX")
                        nc.scalar.activation(out=ex[:], in_=ps[:], func=AF.Exp,
                                             scale=SCALE, accum_out=zsum[:])
                        nc.vector.reciprocal(out=zsum[:], in_=zsum[:])
                        ab = ap_.tile([128, 77], BF16, tag="abx")
                        nc.vector.tensor_scalar_mul(out=ab[:], in0=ex[:],
                                                    scalar1=zsum[:])
                        paT = psp.tile([77, 128], BF16, tag="ps")
                        nc.tensor.transpose(paT[:], ab[:], identb[:])
                        aT = ap_.tile([77, 128], BF16, tag="aTx")
                        nc.scalar.copy(out=aT[:], in_=paT[:])
                        po = psp.tile([DH, 128], F32, tag="ps")
                        nc.tensor.matmul(po[:], vctx[:, DH * h:DH * (h + 1)],
                                         aT[:], start=True, stop=True)
                        nc.scalar.copy(out=oT[:, h, 128 * w:128 * (w + 1)],
                                       in_=po[:])
                for w in range(HALFW):
                    wg = wlo + w
                    wo_residual(psp, oT, wo, w, xw[:, w, :], bo_b)
                    nc.sync.dma_start(out=x_dram[128 * wg:128 * (wg + 1), :],
                                      in_=xw[:, w, :])
                    if debug:
                        nc.sync.dma_start(
                            out=dbg["x2"][:].rearrange(
                                "r t c -> (r t) c")[128 * wg:128 * (wg + 1), :],
                            in_=xw[:, w, :])

        temporal("t2", "t2", first=False)

        # =====================================================================
        # GEGLU FFN (slices of 3 windows)
        # =====================================================================
        with ExitStack() as ph:
            wp = ph.enter_context(tc.tile_pool(name="wpF", bufs=1))
            zp = ph.enter_context(tc.tile_pool(name="zpF", bufs=1))
            hp = ph.enter_context(tc.tile_pool(name="hpF", bufs=2))
            psp = ph.enter_context(tc.tile_pool(name="psF", bufs=8, space="PSUM"))

            w1 = wp.tile([128, CHUNKS, 2 * FFI], BF16, tag="w1")
            nc.sync.dma_start(out=w1[:],
                              in_=wts["ff_w1"][:].rearrange("(a p) n -> p a n",
                                                            p=128))
            w2 = wp.tile([128, FFI // 128, C], BF16, tag="w2")
            nc.sync.dma_start(out=w2[:],
                              in_=wts["ff_w2"][:].rearrange("(a p) n -> p a n",
                                                            p=128))
            b1 = wp.tile([128, 2 * FFI // 128], F32, tag="b1")
            nc.sync.dma_start(out=b1[:],
                              in_=biases["ff_b1"][:].rearrange("(a p) -> p a",
                                                               p=128))
            fb2 = bcast_tile(wp, "ff_b2")

            SW = 3
            NG2 = FFI // 128  # 20
            for s in range(NWIN // SW):
                wlo = s * SW
                ntok = 128 * SW
                xw = zp.tile([128, SW, C], F32, tag="xwf")
                for w in range(SW):
                    load_x_window(xw[:, w, :], wlo + w, False)
                zT = zp.tile([128, CHUNKS, ntok], BF16, tag="zTf")
                ln_to_fm(psp, xw, zT, SW)
                hT = hp.tile([128, 2 * NG2, ntok], BF16, tag="hT")
                for co in range(2 * NG2):
                    pt = psp.tile([128, ntok], F32, tag="ps")
                    for ci in range(CHUNKS):
                        nc.tensor.matmul(pt[:], w1[:, ci, 128 * co:128 * (co + 1)],
                                         zT[:, ci, :],
                                         start=(ci == 0), stop=(ci == CHUNKS - 1))
                    nc.scalar.activation(out=hT[:, co, :], in_=pt[:],
                                         func=AF.Identity,
                                         bias=b1[:, co:co + 1], scale=1.0)
                uT = hp.tile([128, NG2, ntok], BF16, tag="uT")
                for co in range(NG2):
                    gl = hp.tile([128, ntok], BF16, tag="gelu")
                    nc.scalar.activation(out=gl[:], in_=hT[:, NG2 + co, :],
                                         func=AF.Gelu)
                    nc.vector.tensor_mul(out=uT[:, co, :], in0=hT[:, co, :],
                                         in1=gl[:])
                for w in range(SW):
                    wg = wlo + w
                    for (o, n) in nsplits(C):
                        pw = psp.tile([128, 512], F32, tag="ps")
                        for ci in range(NG2):
                            nc.tensor.matmul(pw[:, :n],
                                             uT[:, ci, 128 * w:128 * (w + 1)],
                                             w2[:, ci, o:o + n],
                                             start=(ci == 0), stop=(ci == NG2 - 1))
                        nc.vector.tensor_add(out=xw[:, w, o:o + n], in0=pw[:, :n],
                                             in1=xw[:, w, o:o + n])
                    nc.vector.tensor_add(out=xw[:, w, :], in0=xw[:, w, :],
                                         in1=fb2[:])
                    nc.sync.dma_start(
                        out=out_final[:].rearrange(
                            "r t c -> (r t) c")[128 * wg:128 * (wg + 1), :],
                        in_=xw[:, w, :])

    if not nc.is_finalized():
        nc.finalize()
    return nc


# ----------------------------------------------------------------------------
# host side
# ----------------------------------------------------------------------------

def _bf(a):
    return np.asarray(a, dtype=ml_dtypes.bfloat16)


def prepare_inputs(inputs):
    f = {k: np.asarray(v, dtype=np.float32) for k, v in inputs.items()}
    shared = {}

    def fold(g, b, wname):
        wf = f[wname]
        return f[g][:, None] * wf, f[b] @ wf

    for p, gk, bk_ in (("a1", "g1", "b1"), ("t1", "g4", "b4"),
                       ("t2", "g5", "b5")):
        for kind in ("wq", "wk", "wv"):
            wf, bias = fold(gk, bk_, f"{p}_{kind}")
            shared[f"{p}_{kind}"] = _bf(wf)
            shared[f"{p}_b{kind[1]}"] = bias.astype(np.float32)
    wf, bias = fold("g2", "b2", "a2_wq")
    shared["a2_wq"] = _bf(wf)
    shared["a2_bq"] = bias.astype(np.float32)
    shared["a2_wk"] = _bf(f["a2_wk"])
    shared["a2_wv"] = _bf(f["a2_wv"])
    shared["a2_bk"] = np.zeros(INNER, np.float32)
    shared["a2_bv"] = np.zeros(INNER, np.float32)
    for p in ("a1", "a2", "t1", "t2"):
        shared[f"{p}_wo"] = _bf(
            f[f"{p}_wo"].reshape(HEADS, DH, C).transpose(1, 0, 2))
        shared[f"{p}_bo"] = f[f"{p}_bo"]
    for p in ("t1", "t2"):
        shared[f"{p}_rkT"] = _bf(f[f"{p}_rk"].T)
        rv = f[f"{p}_rv"]
        rvs = np.zeros((16, T, DH), np.float32)
        for t in range(T):
            for j in range(16):
                rvs[j, t] = rv[j - t + MAXREL]
        shared[f"{p}_rvs"] = _bf(rvs)
    w1f, b1f = fold("g3", "b3", "ff_w1")
    shared["ff_w1"] = _bf(w1f)
    shared["ff_b1"] = (b1f + f["ff_b1"]).astype(np.float32)
    shared["ff_w2"] = _bf(f["ff_w2"])
    shared["ff_b2"] = f["ff_b2"]
    m = np.zeros((128, 128), np.float32)
    for g in range(8):
        m[16 * g:16 * (g + 1), 16 * g:16 * (g + 1)] = 1.0
    shared["bd_mask"] = m

    x = f["x"]
    ctx = f["context"]
    in_maps = []
    for core in range(NCORES):
        im = dict(shared)
        xs = np.empty((NG, SEQ, C), np.float32)
        for g in range(NG):
            bt = core + 8 * g
            b, t = bt // T, bt % T
            xs[g] = x[b, :, t].reshape(C, SEQ).T
        im["xs_in"] = xs
        im["ctxT"] = _bf(ctx[core // 4].T.copy())
        in_maps.append(im)
    return in_maps


_PROGRAM_CACHE = {}


def run(inputs, debug=False, trace=False):
    key = "dbg" if debug else "plain"
    if key not in _PROGRAM_CACHE:
        _PROGRAM_CACHE[key] = build_program(debug=debug)
    nc = _PROGRAM_CACHE[key]
    in_maps = prepare_inputs(inputs)
    from concourse.bass_utils import run_bass_kernel_spmd
    res = run_bass_kernel_spmd(nc, in_maps, list(range(NCORES)), trace=trace)
    outs = res.results
    full = np.empty((B * H * W, T, C), np.float32)
    for core in range(NCORES):
        full[NR * core:NR * (core + 1)] = outs[core]["out"]
    y = full.reshape(B, H, W, T, C).transpose(0, 4, 3, 1, 2)
    return y, res, outs


def kernel(**inputs):
    y, _, _ = run(inputs)
    return y.astype(np.float32)



# revision 13
# speedup vs baseline: 1.2363x; 1.2363x over previous
"""Trainium2 Bass kernel for BasicTransformerBlockST (spatial/temporal transformer block).

Sharding over 8 NeuronCores:
  Phase A (spatial self-attn): data-parallel over (b,t): core i owns the 4
  groups bt = i + 8g, so every core holds both batches.
  An 8-way on-device AllToAll reshards to (b,h,w)-parallel: core j owns rows
  (b=j//4, hw in [144*(j%4), 144*(j%4+1))), tokens r-major (token = r*16 + t).
  Phases temporal-1, cross-attn, temporal-2, FFN run on that shard.

Matmul operands bf16 (fp32 PSUM accumulation); residual stream, LN and softmax
statistics fp32. Residual stream lives in DRAM between phases.
"""

import sys

sys.path.insert(0, "/opt/trn_rl_repo")

import numpy as np
import ml_dtypes

import concourse.bass as bass
import concourse.bacc as bacc
import concourse.mybir as mybir
import concourse.tile as tile
from concourse.masks import make_identity

F32 = mybir.dt.float32
BF16 = mybir.dt.bfloat16
AF = mybir.ActivationFunctionType
ALU = mybir.AluOpType

B, C, T, H, W = 2, 640, 16, 24, 24
HEADS, DH = 8, 80
CTXD = 1024
MAXREL = 16
NREL = 2 * MAXREL + 1          # 33
FFI = 4 * C                    # 2560
INNER = HEADS * DH             # 640
SCALE = DH ** -0.5
EPS = 1e-5

NCORES = 8
NG = 4                         # spatial groups per core
SEQ = H * W                    # 576
NR = (B * H * W) // NCORES     # 144 rows per core
TOK = NR * T                   # 2304 tokens per core
NWIN = TOK // 128              # 18
GW = 5                         # windows per padded spatial group
CHUNKS = C // 128              # 5
CTXCH = CTXD // 128            # 8
HALFW = NWIN // 2              # 9 windows per temporal half
HR = NR // 2                   # 72 rows per half


def nsplits(n, cap=512):
    out, o = [], 0
    while o < n:
        out.append((o, min(cap, n - o)))
        o += min(cap, n - o)
    return out


def build_program(debug=False, marks=None):
    nc = bacc.Bacc(None, target_bir_lowering=False)

    def mark(name):
        if marks is not None:
            marks.append((name, nc.next_id()))

    xs_in = nc.dram_tensor("xs_in", [NG, SEQ, C], F32, kind="ExternalInput")
    ctxT_in = nc.dram_tensor("ctxT", [CTXD, 77], BF16, kind="ExternalInput")

    def win(name, shape, dt=BF16):
        return nc.dram_tensor(name, shape, dt, kind="ExternalInput")

    wts, biases = {}, {}
    for p in ("a1", "a2", "t1", "t2"):
        cin = CTXD if p == "a2" else C
        wts[f"{p}_wq"] = win(f"{p}_wq", [C, INNER])
        wts[f"{p}_wk"] = win(f"{p}_wk", [cin, INNER])
        wts[f"{p}_wv"] = win(f"{p}_wv", [cin, INNER])
        wts[f"{p}_wo"] = win(f"{p}_wo", [DH, HEADS, C])
        biases[f"{p}_bq"] = win(f"{p}_bq", [INNER], F32)
        biases[f"{p}_bk"] = win(f"{p}_bk", [INNER], F32)
        biases[f"{p}_bv"] = win(f"{p}_bv", [INNER], F32)
        biases[f"{p}_bo"] = win(f"{p}_bo", [C], F32)
    for p in ("t1", "t2"):
        wts[f"{p}_rkT"] = win(f"{p}_rkT", [DH, NREL])
        wts[f"{p}_rvs"] = win(f"{p}_rvs", [16, T, DH])  # rvs[j,t,d]=rv[j-t+16,d]
    wts["ff_w1"] = win("ff_w1", [C, 2 * FFI])
    wts["ff_w2"] = win("ff_w2", [FFI, C])
    biases["ff_b1"] = win("ff_b1", [2 * FFI], F32)
    biases["ff_b2"] = win("ff_b2", [C], F32)
    bd_mask = win("bd_mask", [128, 128], F32)

    out_final = nc.dram_tensor("out", [NR, T, C], F32, kind="ExternalOutput")
    dbg = {}
    if debug:
        dbg["a"] = nc.dram_tensor("dbg_a", [NG, SEQ, C], F32, kind="ExternalOutput")
        for nm in ("t1", "x2", "t2"):
            dbg[nm] = nc.dram_tensor(f"dbg_{nm}", [NR, T, C], F32,
                                     kind="ExternalOutput")

    a2a_in = nc.dram_tensor("a2a_in", [NCORES, NR, 2, C], F32)
    a2a_out = nc.dram_tensor("a2a_out", [NCORES, NR, 2, C], F32)
    x_dram = nc.dram_tensor("x_dram", [TOK, C], F32)
    sim2_dram = nc.dram_tensor("sim2_dram", [TOK, HEADS, 16], BF16)
    groups = [[0, 1, 2, 3, 4, 5, 6, 7]]

    from contextlib import ExitStack

    with tile.TileContext(nc) as tc, ExitStack() as top:
        const = top.enter_context(tc.tile_pool(name="const", bufs=1))
        ident = const.tile([128, 128], F32)
        make_identity(nc, ident)
        identb = const.tile([128, 128], BF16)
        make_identity(nc, identb)
        eps_t = const.tile([128, 1], F32)
        nc.vector.memset(eps_t[:], EPS)
        mask = const.tile([128, 128], F32)
        nc.sync.dma_start(out=mask[:], in_=bd_mask[:, :])
        small = top.enter_context(tc.tile_pool(name="small", bufs=4))
        zscr = top.enter_context(tc.tile_pool(name="zscr", bufs=2))

        def bcast_tile(wp, name, n=C):
            t = wp.tile([128, n], F32, tag=f"bc_{name}")
            src = biases[name][:]
            bc = bass.AP(tensor=src.tensor, offset=src.offset,
                         ap=[[0, 128], [1, n]])
            nc.gpsimd.dma_start(out=t[:], in_=bc)
            return t

        # ---------------- shared helpers ----------------
        _rr = {"i": 0}

        def rr_copy(out, in_):
            """PSUM->SBUF copy alternating DVE/ACT (Pool has no PSUM port)."""
            i = _rr["i"]
            _rr["i"] = (i + 1) % 2
            if i == 0:
                nc.vector.tensor_scalar_mul(out=out, in0=in_, scalar1=1.0)
            else:
                nc.scalar.copy(out=out, in_=in_)

        def ln_to_fm(psp, x_ap, zT_tile, nw):
            """LN over channels + transpose: x [128,nw,640] f32 ->
            zT [128,CHUNKS,nw*128] bf16 feature-major (normalized, no g/b)."""
            for w in range(nw):
                x = x_ap[:, w, :]
                st = small.tile([128, CHUNKS, 6], F32, tag="bnst")
                for s in range(CHUNKS):
                    nc.vector.bn_stats(out=st[:, s, :],
                                       in_=x[:, 128 * s:128 * (s + 1)])
                mv = small.tile([128, 2], F32, tag="bnmv")
                nc.vector.bn_aggr(out=mv[:], in_=st[:])
                rstd = small.tile([128, 1], F32, tag="rstd")
                nc.scalar.activation(out=rstd[:], in_=mv[:, 1:2], func=AF.Sqrt,
                                     bias=eps_t[:], scale=1.0)
                nc.vector.reciprocal(out=rstd[:], in_=rstd[:])
                zs = zscr.tile([128, C], BF16, tag="zs")
                nc.vector.tensor_scalar(
                    out=zs[:], in0=x, scalar1=mv[:, 0:1], scalar2=rstd[:],
                    op0=ALU.subtract, op1=ALU.mult)
                for c in range(CHUNKS):
                    pt = psp.tile([128, 128], BF16, tag="ps")
                    nc.tensor.transpose(pt[:], zs[:, 128 * c:128 * (c + 1)],
                                        identb[:])
                    rr_copy(zT_tile[:, c, 128 * w:128 * (w + 1)], pt[:])

        def proj_heads(psp, zT, w_sb, out_tile, ntok, bias=None,
                       cin_chunks=CHUNKS):
            """per-head feature-major projection: out [80, HEADS, ntok] bf16."""
            for h in range(HEADS):
                for (o, n) in nsplits(ntok):
                    pt = psp.tile([128, 512], F32, tag="ps")
                    for ci in range(cin_chunks):
                        nc.tensor.matmul(pt[:DH, :n],
                                         w_sb[:, ci, DH * h:DH * (h + 1)],
                                         zT[:, ci, o:o + n],
                                         start=(ci == 0),
                                         stop=(ci == cin_chunks - 1))
                    if bias is not None:
                        nc.vector.tensor_scalar_add(out=out_tile[:, h, o:o + n],
                                                    in0=pt[:DH, :n],
                                                    scalar1=bias[:, h:h + 1])
                    else:
                        nc.vector.tensor_scalar_mul(out=out_tile[:, h, o:o + n],
                                                    in0=pt[:DH, :n], scalar1=1.0)

        def proj_tm(psp, zT, w_sb, out_tile, tok_chunks, badd=None):
            """token-major: out[tok, 640]; lhsT = zT[:,ci,toks], rhs = W."""
            for (w, p, toff) in tok_chunks:
                for (o, n) in nsplits(C):
                    pt = psp.tile([128, 512], F32, tag="ps")
                    for ci in range(CHUNKS):
                        nc.tensor.matmul(pt[:p, :n],
                                         zT[:, ci, toff:toff + p],
                                         w_sb[:, ci, o:o + n],
                                         start=(ci == 0), stop=(ci == CHUNKS - 1))
                    if badd is not None:
                        nc.vector.tensor_add(out=out_tile[:p, w, o:o + n],
                                             in0=pt[:p, :n],
                                             in1=badd[:p, o:o + n])
                    else:
                        nc.scalar.copy(out=out_tile[:p, w, o:o + n],
                                       in_=pt[:p, :n])

        def wo_residual(psp, oT, wo, w, resid_ap, bo):
            """by-head wo projection + bias + residual-add into resid_ap."""
            mp = resid_ap.shape[0]
            for (o, n) in nsplits(C):
                pw = psp.tile([128, 512], F32, tag="ps")
                for h in range(HEADS):
                    nc.tensor.matmul(pw[:mp, :n],
                                     oT[:, h, 128 * w:128 * w + mp],
                                     wo[:, h, o:o + n],
                                     start=(h == 0), stop=(h == HEADS - 1))
                nc.vector.tensor_add(out=resid_ap[:, o:o + n], in0=pw[:mp, :n],
                                     in1=resid_ap[:, o:o + n])
            nc.vector.tensor_add(out=resid_ap[:], in0=resid_ap[:], in1=bo[:mp, :])

        def load_w_cin(wp, name, cin):
            t = wp.tile([128, cin // 128, wts[name].shape[-1]], BF16, tag=name[3:])
            nc.sync.dma_start(out=t[:],
                              in_=wts[name][:].rearrange("(a p) n -> p a n", p=128))
            return t

        def load_wo(wp, name):
            t = wp.tile([DH, HEADS, C], BF16, tag="wo")
            nc.sync.dma_start(out=t[:], in_=wts[name][:])
            return t

        def load_bias_h(wp, name):
            t = wp.tile([DH, HEADS], F32, tag=name[3:] + "b")
            nc.sync.dma_start(out=t[:],
                              in_=biases[name][:].rearrange("(h p) -> p h", p=DH))
            return t

        mark("A")
        # =====================================================================
        # PHASE A: spatial self-attention, per (b,t) group.
        # S^T layout: scores computed k-major (softmax along partitions),
        # Z via ones-column appended to V, normalization deferred to the
        # PSUM->SBUF move of the AV output (per-column scale via K=1 matmul
        # broadcast of 1/Z). No A-transposes, no k/v biases (bk dropped as
        # softmax-invariant; bv folded into bo on host).
        # =====================================================================
        KWIN = [(128 * i, 128 if i < 4 else 64) for i in range(GW)]

        with ExitStack() as ph:
            wp = ph.enter_context(tc.tile_pool(name="wpA", bufs=1))
            zp = ph.enter_context(tc.tile_pool(name="zpA", bufs=2))
            qp = ph.enter_context(tc.tile_pool(name="qpA", bufs=1))
            ep = ph.enter_context(tc.tile_pool(name="epA", bufs=1))
            op_ = ph.enter_context(tc.tile_pool(name="opA", bufs=1))
            psp = ph.enter_context(tc.tile_pool(name="psA", bufs=3, space="PSUM"))

            wq = load_w_cin(wp, "a1_wq", C)
            wk = load_w_cin(wp, "a1_wk", C)
            wv = load_w_cin(wp, "a1_wv", C)
            wo = load_wo(wp, "a1_wo")
            bq = load_bias_h(wp, "a1_bq")
            bo_b = bcast_tile(wp, "a1_bo")
            onesb = wp.tile([1, DH], BF16, tag="onesb")
            nc.vector.memset(onesb[:], 1.0)

            def proj_qk_m(zT, w_sb, out_tile, bias=None):
                """feature-major per-head projection, tails merged:
                out [80, HEADS, 576]."""
                pt_t = psp.tile([128, 512], F32, tag="pst", bufs=2)
                for h in range(HEADS):
                    pt = psp.tile([128, 512], F32, tag="ps")
                    for ci in range(CHUNKS):
                        nc.tensor.matmul(pt[:DH, :],
                                         w_sb[:, ci, DH * h:DH * (h + 1)],
                                         zT[:, ci, 0:512],
                                         start=(ci == 0), stop=(ci == CHUNKS - 1))
                        nc.tensor.matmul(pt_t[:DH, 64 * h:64 * h + 64],
                                         w_sb[:, ci, DH * h:DH * (h + 1)],
                                         zT[:, ci, 512:576],
                                         start=(ci == 0), stop=(ci == CHUNKS - 1))
                    if bias is not None:
                        nc.vector.tensor_scalar_add(out=out_tile[:, h, 0:512],
                                                    in0=pt[:DH, :],
                                                    scalar1=bias[:, h:h + 1])
                    else:
                        rr_copy(out_tile[:, h, 0:512], pt[:DH, :])
                tview = bass.AP(tensor=out_tile.tensor,
                                offset=out_tile[:, :, :].offset + 512,
                                ap=[list(out_tile[:, :, :].ap[0]),
                                    [SEQ, HEADS], [1, 64]])
                pv = pt_t[0:DH, :]
                if bias is not None:
                    bv_ = bass.AP(tensor=bias.tensor, offset=bias[:, :].offset,
                                  ap=[list(bias[:, :].ap[0]), [1, HEADS],
                                      [0, 64]])
                    nc.vector.tensor_tensor(out=tview, in0=pv, in1=bv_,
                                            op=ALU.add)
                else:
                    nc.vector.tensor_scalar_mul(out=tview, in0=pv, scalar1=1.0)

            def proj_v81(zT, w_sb, v_tile):
                """token-major V, per-head 97-wide: cols 0:80 = v, 96 = ones
                (Z lands on psum partition 96, a legal aligned read)."""
                for (w, (ko, kp_)) in enumerate(KWIN):
                    for (o, nh) in ((0, 6), (480, 2)):
                        pt = psp.tile([128, 512], F32, tag="ps")
                        for ci in range(CHUNKS):
                            nc.tensor.matmul(pt[:kp_, :80 * nh],
                                             zT[:, ci, ko:ko + kp_],
                                             w_sb[:, ci, o:o + 80 * nh],
                                             start=(ci == 0),
                                             stop=(ci == CHUNKS - 1))
                        dst = bass.AP(tensor=v_tile.tensor,
                                      offset=v_tile[:, w, :, :].offset
                                      + 97 * (o // 80),
                                      ap=[list(v_tile[:, :, :, :].ap[0]),
                                          [97, nh], [1, 80]])
                        rr_copy(dst, pt[:, 0:80 * nh])

            for g in range(NG):
                xg = zp.tile([128, GW, C], F32, tag="xa")
                nc.sync.dma_start(out=xg[:, 0:4, :],
                                  in_=xs_in[g, 0:512, :].rearrange(
                                      "(a p) c -> p a c", p=128))
                nc.sync.dma_start(out=xg[:64, 4, :], in_=xs_in[g, 512:576, :])
                nc.vector.memset(xg[64:128, 4, :], 0.0)

                zT = zp.tile([128, CHUNKS, GW * 128], BF16, tag="zTa")
                ln_to_fm(psp, xg, zT, GW)

                qT = qp.tile([DH, HEADS, SEQ], BF16, tag="qa")
                kT = qp.tile([DH, HEADS, SEQ], BF16, tag="ka")
                proj_qk_m(zT, wq, qT, bias=bq)
                proj_qk_m(zT, wk, kT, bias=None)
                v = qp.tile([128, GW, HEADS, 97], BF16, tag="va")
                nc.vector.memset(v[:, :, :, 80:96], 0.0)
                nc.vector.memset(v[:, :, :, 96:97], 1.0)
                proj_v81(zT, wv, v)

                # scores S^T = K^T Q per k-window, exp; tails merged over heads
                em = ep.tile([128, GW, HEADS, 512], BF16, tag="em")
                et = ep.tile([128, GW, HEADS * 64], BF16, tag="et")
                for (kw, (ko, kp_)) in enumerate(KWIN):
                    ptt = psp.tile([128, 512], F32, tag="pst", bufs=2)
                    for h in range(HEADS):
                        pt = psp.tile([128, 512], F32, tag="ps")
                        nc.tensor.matmul(pt[:kp_, :], kT[:, h, ko:ko + kp_],
                                         qT[:, h, 0:512],
                                         start=True, stop=True)
                        nc.tensor.matmul(ptt[:kp_, 64 * h:64 * h + 64],
                                         kT[:, h, ko:ko + kp_],
                                         qT[:, h, 512:576],
                                         start=True, stop=True)
                        nc.scalar.activation(out=em[:kp_, kw, h, :],
                                             in_=pt[:kp_, :],
                                             func=AF.Exp, scale=SCALE)
                    nc.scalar.activation(out=et[:kp_, kw, :], in_=ptt[:kp_, :],
                                         func=AF.Exp, scale=SCALE)

                # AV with ones-column -> Z in row 80; normalize on PSUM->SBUF
                oT = op_.tile([DH, HEADS, SEQ], BF16, tag="oa")
                pot = psp.tile([128, 512], F32, tag="pot", bufs=1)
                for h in range(HEADS):
                    po = psp.tile([128, 512], F32, tag="po", bufs=2)
                    for (kw, (ko, kp_)) in enumerate(KWIN):
                        nc.tensor.matmul(po[0:97, :], v[0:kp_, kw, h, :],
                                         em[0:kp_, kw, h, :],
                                         start=(kw == 0), stop=(kw == GW - 1))
                        nc.tensor.matmul(pot[0:97, 64 * h:64 * h + 64],
                                         v[0:kp_, kw, h, :],
                                         et[0:kp_, kw, 64 * h:64 * h + 64],
                                         start=(kw == 0), stop=(kw == GW - 1))
                    zrm = small.tile([1, 512], BF16, tag="zrm")
                    with nc.allow_low_precision(reason="softmax 1/Z in bf16"):
                        nc.vector.reciprocal(out=zrm[:], in_=po[96:97, :])
                    zpm = psp.tile([128, 512], F32, tag="ps")
                    nc.tensor.matmul(zpm[0:DH, :], onesb[:, :], zrm[:, :],
                                     start=True, stop=True)
                    zbc = small.tile([DH, 512], BF16, tag="zbc")
                    rr_copy(zbc[:, :], zpm[0:DH, :])
                    nc.vector.tensor_tensor(out=oT[:, h, 0:512],
                                            in0=po[0:DH, :], in1=zbc[:, :],
                                            op=ALU.mult)
                # merged tails: Z on partition 96 of pot
                zrt = small.tile([1, 512], BF16, tag="zrt")
                with nc.allow_low_precision(reason="softmax 1/Z in bf16"):
                    nc.vector.reciprocal(out=zrt[:], in_=pot[96:97, :])
                zpt = psp.tile([128, 512], F32, tag="ps")
                nc.tensor.matmul(zpt[0:DH, :], onesb[:, :], zrt[:, :],
                                 start=True, stop=True)
                zbt = small.tile([DH, 512], BF16, tag="zbt")
                rr_copy(zbt[:, :], zpt[0:DH, :])
                oview = bass.AP(tensor=oT.tensor, offset=oT[:, :, :].offset + 512,
                                ap=[list(oT[:, :, :].ap[0]), [SEQ, HEADS],
                                    [1, 64]])
                nc.vector.tensor_tensor(out=oview, in0=pot[0:DH, :],
                                        in1=zbt[:, :], op=ALU.mult)

                for (mw, (moff, mp)) in enumerate(KWIN):
                    xn = zp.tile([128, C], F32, tag="xan")
                    nc.scalar.copy(out=xn[:mp, :], in_=xg[:mp, mw, :])
                    wo_residual(psp, oT, wo, mw, xn[:mp, :], bo_b)
                    q0, q1 = moff // NR, (moff + mp - 1) // NR
                    for q in range(q0, q1 + 1):
                        lo, hi = max(moff, NR * q), min(moff + mp, NR * (q + 1))
                        nc.sync.dma_start(
                            out=a2a_in[4 * (g // 2) + q, lo - NR * q:hi - NR * q,
                                       g % 2, :],
                            in_=xn[lo - moff:hi - moff, :])
                    if debug:
                        nc.sync.dma_start(out=dbg["a"][g, moff:moff + mp, :],
                                          in_=xn[:mp, :])

        mark("a2a")
        # =====================================================================
        # AllToAll reshard
        # =====================================================================
        nc.gpsimd.collective_compute("AllToAll", ALU.bypass, replica_groups=groups,
                                     ins=[a2a_in[:]], outs=[a2a_out[:]])

        def load_x_window(dst_ap, wg, first):
            if first:
                base = a2a_out[:]
                src = bass.AP(tensor=base.tensor,
                              offset=base.offset + 8 * wg * 2 * C,
                              ap=[[2 * C, 8], [C, 2], [NR * 2 * C, 8], [1, C]])
            else:
                src = x_dram[128 * wg:128 * (wg + 1), :]
            nc.sync.dma_start(out=dst_ap, in_=src)

        # =====================================================================
        # Temporal attention (t1 / t2)
        # =====================================================================
        def temporal(prefix, dbg_key, first):
            with ExitStack() as ph:
                wp = ph.enter_context(tc.tile_pool(name="wpT", bufs=1))
                zp = ph.enter_context(tc.tile_pool(name="zpT", bufs=1))
                qp = ph.enter_context(tc.tile_pool(name="qpT", bufs=1))
                ap_ = ph.enter_context(tc.tile_pool(name="apT", bufs=2))
                op_ = ph.enter_context(tc.tile_pool(name="opT", bufs=1))
                psp = ph.enter_context(tc.tile_pool(name="psT", bufs=8,
                                                    space="PSUM"))

                wq = load_w_cin(wp, f"{prefix}_wq", C)
                wk = load_w_cin(wp, f"{prefix}_wk", C)
                wv = load_w_cin(wp, f"{prefix}_wv", C)
                wo = load_wo(wp, f"{prefix}_wo")
                bq = load_bias_h(wp, f"{prefix}_bq")
                bk = load_bias_h(wp, f"{prefix}_bk")
                bv_b = bcast_tile(wp, f"{prefix}_bv")
                bo_b = bcast_tile(wp, f"{prefix}_bo")
                rkT = wp.tile([DH, NREL], BF16, tag="rkT")
                nc.sync.dma_start(out=rkT[:], in_=wts[f"{prefix}_rkT"][:])
                rvs = wp.tile([16, T, DH], BF16, tag="rvs")
                nc.sync.dma_start(out=rvs[:], in_=wts[f"{prefix}_rvs"][:])

                for half in range(2):
                    wlo = half * HALFW
                    ntok = 128 * HALFW  # 1152
                    xw = zp.tile([128, HALFW, C], F32, tag="xw")
                    for w in range(HALFW):
                        load_x_window(xw[:, w, :], wlo + w, first)
                    zT = zp.tile([128, CHUNKS, ntok], BF16, tag="zTt")
                    ln_to_fm(psp, xw, zT, HALFW)

                    qT = qp.tile([DH, HEADS, ntok], BF16, tag="qt")
                    kT = qp.tile([DH, HEADS, ntok], BF16, tag="kt")
                    proj_heads(psp, zT, wq, qT, ntok, bias=bq)
                    proj_heads(psp, zT, wk, kT, ntok, bias=bk)
                    v = qp.tile([128, HALFW, C], BF16, tag="vt")
                    proj_tm(psp, zT, wv, v,
                            [(w, 128, 128 * w) for w in range(HALFW)],
                            badd=bv_b)

                    # rel-pos scores P^T = rk . q^T; shear-transpose into
                    # sim2 token layout, bounce via DRAM.
                    s2byT = ap_.tile([HR, T, HEADS, 16], BF16, tag="s2byT")
                    for h in range(HEADS):
                        pSB = ap_.tile([NREL, ntok], BF16, tag="pSB")
                        for (o, n) in nsplits(ntok):
                            pp = psp.tile([NREL, 512], F32, tag="ps")
                            nc.tensor.matmul(pp[:, :n], rkT[:, :],
                                             qT[:, h, o:o + n],
                                             start=True, stop=True)
                            nc.scalar.copy(out=pSB[:, o:o + n], in_=pp[:, :n])
                        for t in range(T):
                            src = bass.AP(tensor=pSB.tensor,
                                          offset=pSB[:, :].offset + t,
                                          ap=[list(pSB[:, :].ap[0]), [16, HR]])
                            pt = psp.tile([HR, NREL], BF16, tag="ps")
                            nc.tensor.transpose(pt[:], src, identb[:NREL, :NREL])
                            nc.scalar.copy(
                                out=s2byT[:, t, h, :],
                                in_=pt[:, MAXREL - t:2 * MAXREL - t])
                    dst = sim2_dram[:].rearrange("(r t) h j -> r t h j", t=T)
                    nc.sync.dma_start(out=dst[HR * half:HR * half + HR],
                                      in_=s2byT[:])

                    # attention windows
                    oT = op_.tile([DH, HEADS, ntok], BF16, tag="ot")
                    aDT = op_.tile([16, HEADS, ntok], BF16, tag="aDT")
                    for w in range(HALFW):
                        wg = wlo + w
                        s2w = ap_.tile([128, HEADS, 16], BF16, tag="s2w")
                        nc.sync.dma_start(
                            out=s2w[:],
                            in_=sim2_dram[128 * wg:128 * (wg + 1), :, :])
                        aG = ap_.tile([128, HEADS, 128], BF16, tag="aG")
                        for h in range(HEADS):
                            ps = psp.tile([128, 128], F32, tag="ps")
                            nc.tensor.matmul(ps[:],
                                             qT[:, h, 128 * w:128 * (w + 1)],
                                             kT[:, h, 128 * w:128 * (w + 1)],
                                             start=True, stop=True)
                            s2rep = bass.AP(
                                tensor=s2w.tensor,
                                offset=s2w[:, h, :].offset,
                                ap=[list(s2w[:, :, :].ap[0]), [0, 8], [1, 16]])
                            tmp = ap_.tile([128, 128], F32, tag="tmpst")
                            nc.vector.scalar_tensor_tensor(
                                out=tmp[:], in0=ps[:], scalar=1.0, in1=s2rep,
                                op0=ALU.mult, op1=ALU.add)
                            exv = ap_.tile([128, 128], F32, tag="exv")
                            nc.scalar.activation(out=exv[:], in_=tmp[:],
                                                 func=AF.Exp, scale=SCALE)
                            zsum = small.tile([128, 1], F32, tag="zsT")
                            nc.vector.scalar_tensor_tensor(
                                out=aG[:, h, :], in0=exv[:], scalar=1.0,
                                in1=mask[:], op0=ALU.mult, op1=ALU.mult,
                                accum_out=zsum[:])
                            nc.vector.reciprocal(out=zsum[:], in_=zsum[:])
                            nc.vector.tensor_scalar_mul(out=aG[:, h, :],
                                                        in0=aG[:, h, :],
                                                        scalar1=zsum[:])
                        # within-row diag blocks: off-diag of aG is zero, so
                        # aD[p,h,j] = sum_g' aG[p,h,16g'+j]
                        aD = ap_.tile([128, HEADS, 16], F32, tag="aD")
                        agv = bass.AP(
                            tensor=aG.tensor, offset=aG[:, :, :].offset,
                            ap=[list(aG[:, :, :].ap[0]), [128, HEADS],
                                [1, 16], [16, 8]])
                        nc.vector.tensor_reduce(
                            out=aD[:], in_=agv, axis=mybir.AxisListType.X,
                            op=ALU.add)
                        for h in range(HEADS):
                            paT = psp.tile([128, 128], BF16, tag="ps")
                            nc.tensor.transpose(paT[:], aG[:, h, :], identb[:])
                            aTs = ap_.tile([128, 128], BF16, tag="aTs")
                            nc.scalar.copy(out=aTs[:], in_=paT[:])
                            po = psp.tile([DH, 128], F32, tag="ps")
                            nc.tensor.matmul(po[:], v[:, w, DH * h:DH * (h + 1)],
                                             aTs[:], start=True, stop=True)
                            nc.scalar.copy(out=oT[:, h, 128 * w:128 * (w + 1)],
                                           in_=po[:])
                            pd = psp.tile([16, 128], F32, tag="ps")
                            nc.tensor.transpose(pd[:], aD[:, h, :], ident[:])
                            nc.scalar.copy(out=aDT[:, h, 128 * w:128 * (w + 1)],
                                           in_=pd[:])
                    # o2: per (t, h): lhsT = pre-shifted rv slice, rhs = aDT cols
                    for t in range(T):
                        for h in range(HEADS):
                            rhs = bass.AP(tensor=aDT.tensor,
                                          offset=aDT[:, h, :].offset + t,
                                          ap=[list(aDT[:, :, :].ap[0]), [16, HR]])
                            p2 = psp.tile([DH, HR], F32, tag="ps")
                            nc.tensor.matmul(p2[:], rvs[:, t, :], rhs,
                                             start=True, stop=True)
                            dstp = bass.AP(tensor=oT.tensor,
                                           offset=oT[:, h, :].offset + t,
                                           ap=[list(oT[:, :, :].ap[0]), [16, HR]])
                            nc.vector.tensor_add(out=dstp, in0=p2[:], in1=dstp)
                    # wo + residual, store to x_dram
                    for w in range(HALFW):
                        wg = wlo + w
                        wo_residual(psp, oT, wo, w, xw[:, w, :], bo_b)
                        nc.sync.dma_start(out=x_dram[128 * wg:128 * (wg + 1), :],
                                          in_=xw[:, w, :])
                        if debug:
                            nc.sync.dma_start(
                                out=dbg[dbg_key][:].rearrange(
                                    "r t c -> (r t) c")[128 * wg:128 * (wg + 1), :],
                                in_=xw[:, w, :])

        mark("t1")
        temporal("t1", "t1", first=True)

        mark("X")
        # =====================================================================
        # Cross-attention (processed in halves)
        # =====================================================================
        with ExitStack() as ph:
            wp = ph.enter_context(tc.tile_pool(name="wpX", bufs=1))
            zp = ph.enter_context(tc.tile_pool(name="zpX", bufs=2))
            qp = ph.enter_context(tc.tile_pool(name="qpX", bufs=1))
            ap_ = ph.enter_context(tc.tile_pool(name="apX", bufs=2))
            op_ = ph.enter_context(tc.tile_pool(name="opX", bufs=2))
            psp = ph.enter_context(tc.tile_pool(name="psX", bufs=8, space="PSUM"))

            wq = load_w_cin(wp, "a2_wq", C)
            wkc = load_w_cin(wp, "a2_wk", CTXD)
            wvc = load_w_cin(wp, "a2_wv", CTXD)
            wo = load_wo(wp, "a2_wo")
            bq = load_bias_h(wp, "a2_bq")
            bo_b = bcast_tile(wp, "a2_bo")

            ctx_sb = wp.tile([128, CTXCH, 77], BF16, tag="ctx")
            nc.sync.dma_start(out=ctx_sb[:],
                              in_=ctxT_in[:].rearrange("(a p) m -> p a m", p=128))
            kctxT = wp.tile([DH, HEADS, 77], BF16, tag="kctx")
            for h in range(HEADS):
                pt = psp.tile([128, 77], F32, tag="ps")
                for ci in range(CTXCH):
                    nc.tensor.matmul(pt[:DH, :], wkc[:, ci, DH * h:DH * (h + 1)],
                                     ctx_sb[:, ci, :],
                                     start=(ci == 0), stop=(ci == CTXCH - 1))
                nc.scalar.copy(out=kctxT[:, h, :], in_=pt[:DH, :])
            vctx = wp.tile([77, C], BF16, tag="vctx")
            for (o, n) in nsplits(C):
                pt = psp.tile([77, 512], F32, tag="ps")
                for ci in range(CTXCH):
                    nc.tensor.matmul(pt[:, :n], ctx_sb[:, ci, :],
                                     wvc[:, ci, o:o + n],
                                     start=(ci == 0), stop=(ci == CTXCH - 1))
                nc.scalar.copy(out=vctx[:, o:o + n], in_=pt[:, :n])

            for half in range(2):
                wlo = half * HALFW
                ntok = 128 * HALFW
                xw = zp.tile([128, HALFW, C], F32, tag="xwx")
                for w in range(HALFW):
                    load_x_window(xw[:, w, :], wlo + w, False)
                zT = zp.tile([128, CHUNKS, ntok], BF16, tag="zTx")
                ln_to_fm(psp, xw, zT, HALFW)
                qT = qp.tile([DH, HEADS, ntok], BF16, tag="qx")
                proj_heads(psp, zT, wq, qT, ntok, bias=bq)

                oT = op_.tile([DH, HEADS, ntok], BF16, tag="ox")
                for w in range(HALFW):
                    for h in range(HEADS):
                        ps = psp.tile([128, 77], F32, tag="ps")
                        nc.tensor.matmul(ps[:], qT[:, h, 128 * w:128 * (w + 1)],
                                         kctxT[:, h, :], start=True, stop=True)
                        ex = ap_.tile([128, 77], F32, tag="exx")
                        zsum = small.tile([128, 1], F32, tag="zsX")
                        nc.scalar.activation(out=ex[:], in_=ps[:], func=AF.Exp,
                                             scale=SCALE, accum_out=zsum[:])
                        nc.vector.reciprocal(out=zsum[:], in_=zsum[:])
                        ab = ap_.tile([128, 77], BF16, tag="abx")
                        nc.vector.tensor_scalar_mul(out=ab[:], in0=ex[:],
                                                    scalar1=zsum[:])
                        paT = psp.tile([77, 128], BF16, tag="ps")
                        nc.tensor.transpose(paT[:], ab[:], identb[:])
                        aT = ap_.tile([77, 128], BF16, tag="aTx")
                        nc.scalar.copy(out=aT[:], in_=paT[:])
                        po = psp.tile([DH, 128], F32, tag="ps")
                        nc.tensor.matmul(po[:], vctx[:, DH * h:DH * (h + 1)],
                                         aT[:], start=True, stop=True)
                        nc.scalar.copy(out=oT[:, h, 128 * w:128 * (w + 1)],
                                       in_=po[:])
                for w in range(HALFW):
                    wg = wlo + w
                    wo_residual(psp, oT, wo, w, xw[:, w, :], bo_b)
                    nc.sync.dma_start(out=x_dram[128 * wg:128 * (wg + 1), :],
                                      in_=xw[:, w, :])
                    if debug:
                        nc.sync.dma_start(
                            out=dbg["x2"][:].rearrange(
                                "r t c -> (r t) c")[128 * wg:128 * (wg + 1), :],
                            in_=xw[:, w, :])

        mark("t2")
        temporal("t2", "t2", first=False)

        mark("FFN")
        # =====================================================================
        # GEGLU FFN (slices of 3 windows)
        # =====================================================================
        with ExitStack() as ph:
            wp = ph.enter_context(tc.tile_pool(name="wpF", bufs=1))
            zp = ph.enter_context(tc.tile_pool(name="zpF", bufs=1))
            hp = ph.enter_context(tc.tile_pool(name="hpF", bufs=2))
            psp = ph.enter_context(tc.tile_pool(name="psF", bufs=8, space="PSUM"))

            w1 = wp.tile([128, CHUNKS, 2 * FFI], BF16, tag="w1")
            nc.sync.dma_start(out=w1[:],
                              in_=wts["ff_w1"][:].rearrange("(a p) n -> p a n",
                                                            p=128))
            w2 = wp.tile([128, FFI // 128, C], BF16, tag="w2")
            nc.sync.dma_start(out=w2[:],
                              in_=wts["ff_w2"][:].rearrange("(a p) n -> p a n",
                                                            p=128))
            b1 = wp.tile([128, 2 * FFI // 128], F32, tag="b1")
            nc.sync.dma_start(out=b1[:],
                              in_=biases["ff_b1"][:].rearrange("(a p) -> p a",
                                                               p=128))
            fb2 = bcast_tile(wp, "ff_b2")

            SW = 3
            NG2 = FFI // 128  # 20
            for s in range(NWIN // SW):
                wlo = s * SW
                ntok = 128 * SW
                xw = zp.tile([128, SW, C], F32, tag="xwf")
                for w in range(SW):
                    load_x_window(xw[:, w, :], wlo + w, False)
                zT = zp.tile([128, CHUNKS, ntok], BF16, tag="zTf")
                ln_to_fm(psp, xw, zT, SW)
                hT = hp.tile([128, 2 * NG2, ntok], BF16, tag="hT")
                for co in range(2 * NG2):
                    pt = psp.tile([128, ntok], F32, tag="ps")
                    for ci in range(CHUNKS):
                        nc.tensor.matmul(pt[:], w1[:, ci, 128 * co:128 * (co + 1)],
                                         zT[:, ci, :],
                                         start=(ci == 0), stop=(ci == CHUNKS - 1))
                    nc.scalar.activation(out=hT[:, co, :], in_=pt[:],
                                         func=AF.Identity,
                                         bias=b1[:, co:co + 1], scale=1.0)
                uT = hp.tile([128, NG2, ntok], BF16, tag="uT")
                for co in range(NG2):
                    gl = hp.tile([128, ntok], BF16, tag="gelu")
                    nc.scalar.activation(out=gl[:], in_=hT[:, NG2 + co, :],
                                         func=AF.Gelu)
                    nc.vector.tensor_mul(out=uT[:, co, :], in0=hT[:, co, :],
                                         in1=gl[:])
                for w in range(SW):
                    wg = wlo + w
                    for (o, n) in nsplits(C):
                        pw = psp.tile([128, 512], F32, tag="ps")
                        for ci in range(NG2):
                            nc.tensor.matmul(pw[:, :n],
                                             uT[:, ci, 128 * w:128 * (w + 1)],
                                             w2[:, ci, o:o + n],
                                             start=(ci == 0), stop=(ci == NG2 - 1))
                        nc.vector.tensor_add(out=xw[:, w, o:o + n], in0=pw[:, :n],
                                             in1=xw[:, w, o:o + n])
                    nc.vector.tensor_add(out=xw[:, w, :], in0=xw[:, w, :],
                                         in1=fb2[:])
                    nc.sync.dma_start(
                        out=out_final[:].rearrange(
                            "r t c -> (r t) c")[128 * wg:128 * (wg + 1), :],
                        in_=xw[:, w, :])

    if not nc.is_finalized():
        nc.finalize()
    return nc


# ----------------------------------------------------------------------------
# host side
# ----------------------------------------------------------------------------

def _bf(a):
    return np.asarray(a, dtype=ml_dtypes.bfloat16)


def prepare_inputs(inputs):
    f = {k: np.asarray(v, dtype=np.float32) for k, v in inputs.items()}
    shared = {}

    def fold(g, b, wname):
        wf = f[wname]
        return f[g][:, None] * wf, f[b] @ wf

    for p, gk, bk_ in (("a1", "g1", "b1"), ("t1", "g4", "b4"),
                       ("t2", "g5", "b5")):
        for kind in ("wq", "wk", "wv"):
            wf, bias = fold(gk, bk_, f"{p}_{kind}")
            shared[f"{p}_{kind}"] = _bf(wf)
            shared[f"{p}_b{kind[1]}"] = bias.astype(np.float32)

    wf, bias = fold("g2", "b2", "a2_wq")
    shared["a2_wq"] = _bf(wf)
    shared["a2_bq"] = bias.astype(np.float32)
    shared["a2_wk"] = _bf(f["a2_wk"])
    shared["a2_wv"] = _bf(f["a2_wv"])
    shared["a2_bk"] = np.zeros(INNER, np.float32)
    shared["a2_bv"] = np.zeros(INNER, np.float32)
    for p in ("a1", "a2", "t1", "t2"):
        shared[f"{p}_wo"] = _bf(
            f[f"{p}_wo"].reshape(HEADS, DH, C).transpose(1, 0, 2))
        shared[f"{p}_bo"] = f[f"{p}_bo"]
    # phase A S^T scheme: bk dropped (softmax-invariant), bv folded into bo
    shared["a1_bo"] = (f["a1_bo"].astype(np.float64)
                       + shared["a1_bv"].astype(np.float64)
                       @ f["a1_wo"].astype(np.float64)).astype(np.float32)
    for p in ("t1", "t2"):
        shared[f"{p}_rkT"] = _bf(f[f"{p}_rk"].T)
        rv = f[f"{p}_rv"]
        rvs = np.zeros((16, T, DH), np.float32)
        for t in range(T):
            for j in range(16):
                rvs[j, t] = rv[j - t + MAXREL]
        shared[f"{p}_rvs"] = _bf(rvs)
    w1f, b1f = fold("g3", "b3", "ff_w1")
    shared["ff_w1"] = _bf(w1f)
    shared["ff_b1"] = (b1f + f["ff_b1"]).astype(np.float32)
    shared["ff_w2"] = _bf(f["ff_w2"])
    shared["ff_b2"] = f["ff_b2"]
    m = np.zeros((128, 128), np.float32)
    for g in range(8):
        m[16 * g:16 * (g + 1), 16 * g:16 * (g + 1)] = 1.0
    shared["bd_mask"] = m

    x = f["x"]
    ctx = f["context"]
    in_maps = []
    for core in range(NCORES):
        im = dict(shared)
        xs = np.empty((NG, SEQ, C), np.float32)
        for g in range(NG):
            bt = core + 8 * g
            b, t = bt // T, bt % T
            xs[g] = x[b, :, t].reshape(C, SEQ).T
        im["xs_in"] = xs
        im["ctxT"] = _bf(ctx[core // 4].T.copy())
        in_maps.append(im)
    return in_maps


_PROGRAM_CACHE = {}


def run(inputs, debug=False, trace=False):
    key = "dbg" if debug else "plain"
    if key not in _PROGRAM_CACHE:
        _PROGRAM_CACHE[key] = build_program(debug=debug)
    nc = _PROGRAM_CACHE[key]
    in_maps = prepare_inputs(inputs)
    from concourse.bass_utils import run_bass_kernel_spmd
    res = run_bass_kernel_spmd(nc, in_maps, list(range(NCORES)), trace=trace)
    outs = res.results
    full = np.empty((B * H * W, T, C), np.float32)
    for core in range(NCORES):
        full[NR * core:NR * (core + 1)] = outs[core]["out"]
    y = full.reshape(B, H, W, T, C).transpose(0, 4, 3, 1, 2)
    return y, res, outs


def kernel(**inputs):
    y, _, _ = run(inputs)
    return y.astype(np.float32)



# revision 23
# speedup vs baseline: 1.5870x; 1.2837x over previous
"""Trainium2 Bass kernel for BasicTransformerBlockST (spatial/temporal transformer block).

Sharding over 8 NeuronCores:
  Phase A (spatial self-attn): data-parallel over (b,t): core i owns the 4
  groups bt = i + 8g, so every core holds both batches.
  An 8-way on-device AllToAll reshards to (b,h,w)-parallel: core j owns rows
  (b=j//4, hw in [144*(j%4), 144*(j%4+1))), tokens r-major (token = r*16 + t).
  Phases temporal-1, cross-attn, temporal-2, FFN run on that shard.

Matmul operands bf16 (fp32 PSUM accumulation); residual stream, LN and softmax
statistics fp32. Residual stream lives in DRAM between phases.
"""

import sys

sys.path.insert(0, "/opt/trn_rl_repo")

import numpy as np
import ml_dtypes

import concourse.bass as bass
import concourse.bacc as bacc
import concourse.mybir as mybir
import concourse.tile as tile
from concourse.masks import make_identity

F32 = mybir.dt.float32
BF16 = mybir.dt.bfloat16
AF = mybir.ActivationFunctionType
ALU = mybir.AluOpType

B, C, T, H, W = 2, 640, 16, 24, 24
HEADS, DH = 8, 80
CTXD = 1024
MAXREL = 16
NREL = 2 * MAXREL + 1          # 33
FFI = 4 * C                    # 2560
INNER = HEADS * DH             # 640
SCALE = DH ** -0.5
EPS = 1e-5

NCORES = 8
NG = 4                         # spatial groups per core
SEQ = H * W                    # 576
NR = (B * H * W) // NCORES     # 144 rows per core
TOK = NR * T                   # 2304 tokens per core
NWIN = TOK // 128              # 18
GW = 5                         # windows per padded spatial group
CHUNKS = C // 128              # 5
CTXCH = CTXD // 128            # 8
HALFW = NWIN // 2              # 9 windows per temporal half
HR = NR // 2                   # 72 rows per half


def nsplits(n, cap=512):
    out, o = [], 0
    while o < n:
        out.append((o, min(cap, n - o)))
        o += min(cap, n - o)
    return out


def build_program(debug=False, marks=None):
    nc = bacc.Bacc(None, target_bir_lowering=False)

    def mark(name):
        if marks is not None:
            marks.append((name, nc.next_id()))

    xs_in = nc.dram_tensor("xs_in", [NG, SEQ, C], F32, kind="ExternalInput")
    ctxT_in = nc.dram_tensor("ctxT", [CTXD, 77], BF16, kind="ExternalInput")

    def win(name, shape, dt=BF16):
        return nc.dram_tensor(name, shape, dt, kind="ExternalInput")

    wts, biases = {}, {}
    for p in ("a1", "a2", "t1", "t2"):
        cin = CTXD if p == "a2" else C
        wts[f"{p}_wq"] = win(f"{p}_wq", [C, INNER])
        wts[f"{p}_wk"] = win(f"{p}_wk", [cin, INNER])
        wts[f"{p}_wv"] = win(f"{p}_wv", [cin, INNER])
        wts[f"{p}_wo"] = win(f"{p}_wo", [DH, HEADS, C])
        biases[f"{p}_bq"] = win(f"{p}_bq", [INNER], F32)
        biases[f"{p}_bk"] = win(f"{p}_bk", [INNER], F32)
        biases[f"{p}_bv"] = win(f"{p}_bv", [INNER], F32)
        biases[f"{p}_bo"] = win(f"{p}_bo", [C], F32)
    for p in ("t1", "t2"):
        wts[f"{p}_rkT"] = win(f"{p}_rkT", [DH, NREL])
        wts[f"{p}_rvB"] = win(f"{p}_rvB", [128, T, DH])  # rvB[16g+t',t,d]=rv[t'-t+16,d]
    wts["ff_w1"] = win("ff_w1", [C, 2 * FFI])
    wts["ff_w2"] = win("ff_w2", [FFI, C])
    biases["ff_b1"] = win("ff_b1", [2 * FFI], F32)
    biases["ff_b2"] = win("ff_b2", [C], F32)
    bd_mask = win("bd_mask", [128, 128])

    out_final = nc.dram_tensor("out", [NR, T, C], F32, kind="ExternalOutput")
    dbg = {}
    if debug:
        dbg["a"] = nc.dram_tensor("dbg_a", [NG, SEQ, C], F32, kind="ExternalOutput")
        for nm in ("t1", "x2", "t2"):
            dbg[nm] = nc.dram_tensor(f"dbg_{nm}", [NR, T, C], F32,
                                     kind="ExternalOutput")
        for nm in ("qT", "kT", "oT"):
            dbg[nm] = nc.dram_tensor(f"dbg_{nm}", [DH, HEADS, 128 * HALFW],
                                     F32, kind="ExternalOutput")
        dbg["vt"] = nc.dram_tensor("dbg_vt", [128, HALFW, C], F32,
                                   kind="ExternalOutput")
        dbg["s2"] = nc.dram_tensor("dbg_s2", [HEADS, 128, 16, HALFW], F32,
                                   kind="ExternalOutput")
        dbg["aU"] = nc.dram_tensor("dbg_aU", [HALFW, 128, 128], F32,
                                   kind="ExternalOutput")

    a2a_in = nc.dram_tensor("a2a_in", [NCORES, NR, 2, C], F32)
    a2a_out = nc.dram_tensor("a2a_out", [NCORES, NR, 2, C], F32)
    x_dram = nc.dram_tensor("x_dram", [TOK, C], F32)
    p_dram = nc.dram_tensor("p_dram", [HEADS, 2, 128, HALFW * NREL], BF16)
    groups = [[0, 1, 2, 3, 4, 5, 6, 7]]

    from contextlib import ExitStack

    with tile.TileContext(nc) as tc, ExitStack() as top:
        const = top.enter_context(tc.tile_pool(name="const", bufs=1))
        ident = const.tile([128, 128], F32)
        make_identity(nc, ident)
        identb = const.tile([128, 128], BF16)
        make_identity(nc, identb)
        eps_t = const.tile([128, 1], F32)
        nc.vector.memset(eps_t[:], EPS)
        maskb = const.tile([128, 128], BF16)
        nc.sync.dma_start(out=maskb[:], in_=bd_mask[:, :])
        small = top.enter_context(tc.tile_pool(name="small", bufs=4))
        zscr = top.enter_context(tc.tile_pool(name="zscr", bufs=2))

        def bcast_tile(wp, name, n=C):
            t = wp.tile([128, n], F32, tag=f"bc_{name}")
            src = biases[name][:]
            bc = bass.AP(tensor=src.tensor, offset=src.offset,
                         ap=[[0, 128], [1, n]])
            nc.gpsimd.dma_start(out=t[:], in_=bc)
            return t

        # ---------------- shared helpers ----------------
        _rr = {"i": 0}

        def rr_copy(out, in_):
            """PSUM->SBUF copy alternating DVE/ACT (Pool has no PSUM port)."""
            i = _rr["i"]
            _rr["i"] = (i + 1) % 2
            if i == 0:
                nc.vector.tensor_scalar_mul(out=out, in0=in_, scalar1=1.0)
            else:
                nc.scalar.copy(out=out, in_=in_)

        def ln_to_fm(psp, x_ap, zT_tile, nw):
            """LN over channels + transpose: x [128,nw,640] f32 ->
            zT [128,CHUNKS,nw*128] bf16 feature-major (normalized, no g/b)."""
            for w in range(nw):
                x = x_ap[:, w, :]
                st = small.tile([128, CHUNKS, 6], F32, tag="bnst")
                for s in range(CHUNKS):
                    nc.vector.bn_stats(out=st[:, s, :],
                                       in_=x[:, 128 * s:128 * (s + 1)])
                mv = small.tile([128, 2], F32, tag="bnmv")
                nc.vector.bn_aggr(out=mv[:], in_=st[:])
                rstd = small.tile([128, 1], F32, tag="rstd")
                nc.scalar.activation(out=rstd[:], in_=mv[:, 1:2], func=AF.Sqrt,
                                     bias=eps_t[:], scale=1.0)
                nc.vector.reciprocal(out=rstd[:], in_=rstd[:])
                zs = zscr.tile([128, C], BF16, tag="zs")
                nc.vector.tensor_scalar(
                    out=zs[:], in0=x, scalar1=mv[:, 0:1], scalar2=rstd[:],
                    op0=ALU.subtract, op1=ALU.mult)
                for c in range(CHUNKS):
                    pt = psp.tile([128, 128], BF16, tag="ps")
                    nc.tensor.transpose(pt[:], zs[:, 128 * c:128 * (c + 1)],
                                        identb[:])
                    rr_copy(zT_tile[:, c, 128 * w:128 * (w + 1)], pt[:])

        def proj_heads(psp, zT, w_sb, out_tile, ntok, bias=None,
                       cin_chunks=CHUNKS):
            """per-head feature-major projection: out [80, HEADS, ntok] bf16."""
            for h in range(HEADS):
                for (o, n) in nsplits(ntok):
                    pt = psp.tile([128, 512], F32, tag="ps")
                    for ci in range(cin_chunks):
                        nc.tensor.matmul(pt[:DH, :n],
                                         w_sb[:, ci, DH * h:DH * (h + 1)],
                                         zT[:, ci, o:o + n],
                                         start=(ci == 0),
                                         stop=(ci == cin_chunks - 1))
                    if bias is not None:
                        nc.vector.tensor_scalar_add(out=out_tile[:, h, o:o + n],
                                                    in0=pt[:DH, :n],
                                                    scalar1=bias[:, h:h + 1])
                    else:
                        nc.vector.tensor_scalar_mul(out=out_tile[:, h, o:o + n],
                                                    in0=pt[:DH, :n], scalar1=1.0)

        def proj_tm(psp, zT, w_sb, out_tile, tok_chunks, badd=None):
            """token-major: out[tok, 640]; lhsT = zT[:,ci,toks], rhs = W."""
            for (w, p, toff) in tok_chunks:
                for (o, n) in nsplits(C):
                    pt = psp.tile([128, 512], F32, tag="ps")
                    for ci in range(CHUNKS):
                        nc.tensor.matmul(pt[:p, :n],
                                         zT[:, ci, toff:toff + p],
                                         w_sb[:, ci, o:o + n],
                                         start=(ci == 0), stop=(ci == CHUNKS - 1))
                    if badd is not None:
                        nc.vector.tensor_add(out=out_tile[:p, w, o:o + n],
                                             in0=pt[:p, :n],
                                             in1=badd[:p, o:o + n])
                    else:
                        nc.scalar.copy(out=out_tile[:p, w, o:o + n],
                                       in_=pt[:p, :n])

        def wo_residual(psp, oT, wo, w, resid_ap, bo):
            """by-head wo projection + bias + residual-add into resid_ap."""
            mp = resid_ap.shape[0]
            for (o, n) in nsplits(C):
                pw = psp.tile([128, 512], F32, tag="ps")
                for h in range(HEADS):
                    nc.tensor.matmul(pw[:mp, :n],
                                     oT[:, h, 128 * w:128 * w + mp],
                                     wo[:, h, o:o + n],
                                     start=(h == 0), stop=(h == HEADS - 1))
                nc.vector.tensor_add(out=resid_ap[:, o:o + n], in0=pw[:mp, :n],
                                     in1=resid_ap[:, o:o + n])
            nc.vector.tensor_add(out=resid_ap[:], in0=resid_ap[:], in1=bo[:mp, :])

        def load_w_cin(wp, name, cin):
            t = wp.tile([128, cin // 128, wts[name].shape[-1]], BF16, tag=name[3:])
            nc.sync.dma_start(out=t[:],
                              in_=wts[name][:].rearrange("(a p) n -> p a n", p=128))
            return t

        def load_wo(wp, name):
            t = wp.tile([DH, HEADS, C], BF16, tag="wo")
            nc.sync.dma_start(out=t[:], in_=wts[name][:])
            return t

        def load_bias_h(wp, name):
            t = wp.tile([DH, HEADS], F32, tag=name[3:] + "b")
            nc.sync.dma_start(out=t[:],
                              in_=biases[name][:].rearrange("(h p) -> p h", p=DH))
            return t

        mark("A")
        # =====================================================================
        # PHASE A: spatial self-attention, per (b,t) group.
        # S^T layout: scores computed k-major (softmax along partitions),
        # Z via ones-column appended to V, normalization deferred to the
        # PSUM->SBUF move of the AV output (per-column scale via K=1 matmul
        # broadcast of 1/Z). No A-transposes, no k/v biases (bk dropped as
        # softmax-invariant; bv folded into bo on host).
        # =====================================================================
        KWIN = [(128 * i, 128 if i < 4 else 64) for i in range(GW)]

        with ExitStack() as ph:
            wp = ph.enter_context(tc.tile_pool(name="wpA", bufs=1))
            zp = ph.enter_context(tc.tile_pool(name="zpA", bufs=2))
            qp = ph.enter_context(tc.tile_pool(name="qpA", bufs=1))
            ep = ph.enter_context(tc.tile_pool(name="epA", bufs=1))
            op_ = ph.enter_context(tc.tile_pool(name="opA", bufs=1))
            psp = ph.enter_context(tc.tile_pool(name="psA", bufs=3, space="PSUM"))

            wq = load_w_cin(wp, "a1_wq", C)
            wk = load_w_cin(wp, "a1_wk", C)
            wv = load_w_cin(wp, "a1_wv", C)
            wo = load_wo(wp, "a1_wo")
            bq = load_bias_h(wp, "a1_bq")
            bo_b = bcast_tile(wp, "a1_bo")
            onesb = wp.tile([1, DH], BF16, tag="onesb")
            nc.vector.memset(onesb[:], 1.0)

            def proj_qk_m(zT, w_sb, out_tile, bias=None):
                """feature-major per-head projection, tails merged:
                out [80, HEADS, 576]."""
                pt_t = psp.tile([128, 512], F32, tag="pst", bufs=2)
                for h in range(HEADS):
                    pt = psp.tile([128, 512], F32, tag="ps")
                    for ci in range(CHUNKS):
                        nc.tensor.matmul(pt[:DH, :],
                                         w_sb[:, ci, DH * h:DH * (h + 1)],
                                         zT[:, ci, 0:512],
                                         start=(ci == 0), stop=(ci == CHUNKS - 1))
                        nc.tensor.matmul(pt_t[:DH, 64 * h:64 * h + 64],
                                         w_sb[:, ci, DH * h:DH * (h + 1)],
                                         zT[:, ci, 512:576],
                                         start=(ci == 0), stop=(ci == CHUNKS - 1))
                    if bias is not None:
                        nc.vector.tensor_scalar_add(out=out_tile[:, h, 0:512],
                                                    in0=pt[:DH, :],
                                                    scalar1=bias[:, h:h + 1])
                    else:
                        rr_copy(out_tile[:, h, 0:512], pt[:DH, :])
                tview = bass.AP(tensor=out_tile.tensor,
                                offset=out_tile[:, :, :].offset + 512,
                                ap=[list(out_tile[:, :, :].ap[0]),
                                    [SEQ, HEADS], [1, 64]])
                pv = pt_t[0:DH, :]
                if bias is not None:
                    bv_ = bass.AP(tensor=bias.tensor, offset=bias[:, :].offset,
                                  ap=[list(bias[:, :].ap[0]), [1, HEADS],
                                      [0, 64]])
                    nc.vector.tensor_tensor(out=tview, in0=pv, in1=bv_,
                                            op=ALU.add)
                else:
                    nc.vector.tensor_scalar_mul(out=tview, in0=pv, scalar1=1.0)

            def proj_v81(zT, w_sb, v_tile):
                """token-major V, per-head 97-wide: cols 0:80 = v, 96 = ones
                (Z lands on psum partition 96, a legal aligned read)."""
                for (w, (ko, kp_)) in enumerate(KWIN):
                    for (o, nh) in ((0, 6), (480, 2)):
                        pt = psp.tile([128, 512], F32, tag="ps")
                        for ci in range(CHUNKS):
                            nc.tensor.matmul(pt[:kp_, :80 * nh],
                                             zT[:, ci, ko:ko + kp_],
                                             w_sb[:, ci, o:o + 80 * nh],
                                             start=(ci == 0),
                                             stop=(ci == CHUNKS - 1))
                        dst = bass.AP(tensor=v_tile.tensor,
                                      offset=v_tile[:, w, :, :].offset
                                      + 97 * (o // 80),
                                      ap=[list(v_tile[:, :, :, :].ap[0]),
                                          [97, nh], [1, 80]])
                        rr_copy(dst, pt[:, 0:80 * nh])

            for g in range(NG):
                xg = zp.tile([128, GW, C], F32, tag="xa")
                nc.sync.dma_start(out=xg[:, 0:4, :],
                                  in_=xs_in[g, 0:512, :].rearrange(
                                      "(a p) c -> p a c", p=128))
                nc.sync.dma_start(out=xg[:64, 4, :], in_=xs_in[g, 512:576, :])
                nc.vector.memset(xg[64:128, 4, :], 0.0)

                zT = zp.tile([128, CHUNKS, GW * 128], BF16, tag="zTa")
                ln_to_fm(psp, xg, zT, GW)

                qT = qp.tile([DH, HEADS, SEQ], BF16, tag="qa")
                kT = qp.tile([DH, HEADS, SEQ], BF16, tag="ka")
                proj_qk_m(zT, wq, qT, bias=bq)
                proj_qk_m(zT, wk, kT, bias=None)
                v = qp.tile([128, GW, HEADS, 97], BF16, tag="va")
                nc.vector.memset(v[:, :, :, 80:96], 0.0)
                nc.vector.memset(v[:, :, :, 96:97], 1.0)
                proj_v81(zT, wv, v)

                # scores S^T = K^T Q per k-window, exp; tails merged over heads
                em = ep.tile([128, GW, HEADS, 512], BF16, tag="em")
                et = ep.tile([128, GW, HEADS * 64], BF16, tag="et")
                for (kw, (ko, kp_)) in enumerate(KWIN):
                    ptt = psp.tile([128, 512], F32, tag="pst", bufs=2)
                    for h in range(HEADS):
                        pt = psp.tile([128, 512], F32, tag="ps")
                        nc.tensor.matmul(pt[:kp_, :], kT[:, h, ko:ko + kp_],
                                         qT[:, h, 0:512],
                                         start=True, stop=True)
                        nc.tensor.matmul(ptt[:kp_, 64 * h:64 * h + 64],
                                         kT[:, h, ko:ko + kp_],
                                         qT[:, h, 512:576],
                                         start=True, stop=True)
                        nc.scalar.activation(out=em[:kp_, kw, h, :],
                                             in_=pt[:kp_, :],
                                             func=AF.Exp, scale=SCALE)
                    nc.scalar.activation(out=et[:kp_, kw, :], in_=ptt[:kp_, :],
                                         func=AF.Exp, scale=SCALE)

                # AV with ones-column -> Z in row 80; normalize on PSUM->SBUF
                oT = op_.tile([DH, HEADS, SEQ], BF16, tag="oa")
                pot = psp.tile([128, 512], F32, tag="pot", bufs=1)
                for h in range(HEADS):
                    po = psp.tile([128, 512], F32, tag="po", bufs=2)
                    for (kw, (ko, kp_)) in enumerate(KWIN):
                        nc.tensor.matmul(po[0:97, :], v[0:kp_, kw, h, :],
                                         em[0:kp_, kw, h, :],
                                         start=(kw == 0), stop=(kw == GW - 1))
                        nc.tensor.matmul(pot[0:97, 64 * h:64 * h + 64],
                                         v[0:kp_, kw, h, :],
                                         et[0:kp_, kw, 64 * h:64 * h + 64],
                                         start=(kw == 0), stop=(kw == GW - 1))
                    zrm = small.tile([1, 512], BF16, tag="zrm")
                    with nc.allow_low_precision(reason="softmax 1/Z in bf16"):
                        nc.vector.reciprocal(out=zrm[:], in_=po[96:97, :])
                    zpm = psp.tile([128, 512], F32, tag="ps")
                    nc.tensor.matmul(zpm[0:DH, :], onesb[:, :], zrm[:, :],
                                     start=True, stop=True)
                    zbc = small.tile([DH, 512], BF16, tag="zbc")
                    rr_copy(zbc[:, :], zpm[0:DH, :])
                    nc.vector.tensor_tensor(out=oT[:, h, 0:512],
                                            in0=po[0:DH, :], in1=zbc[:, :],
                                            op=ALU.mult)
                # merged tails: Z on partition 96 of pot
                zrt = small.tile([1, 512], BF16, tag="zrt")
                with nc.allow_low_precision(reason="softmax 1/Z in bf16"):
                    nc.vector.reciprocal(out=zrt[:], in_=pot[96:97, :])
                zpt = psp.tile([128, 512], F32, tag="ps")
                nc.tensor.matmul(zpt[0:DH, :], onesb[:, :], zrt[:, :],
                                 start=True, stop=True)
                zbt = small.tile([DH, 512], BF16, tag="zbt")
                rr_copy(zbt[:, :], zpt[0:DH, :])
                oview = bass.AP(tensor=oT.tensor, offset=oT[:, :, :].offset + 512,
                                ap=[list(oT[:, :, :].ap[0]), [SEQ, HEADS],
                                    [1, 64]])
                nc.vector.tensor_tensor(out=oview, in0=pot[0:DH, :],
                                        in1=zbt[:, :], op=ALU.mult)

                for (mw, (moff, mp)) in enumerate(KWIN):
                    xn = zp.tile([128, C], F32, tag="xan")
                    nc.scalar.copy(out=xn[:mp, :], in_=xg[:mp, mw, :])
                    wo_residual(psp, oT, wo, mw, xn[:mp, :], bo_b)
                    q0, q1 = moff // NR, (moff + mp - 1) // NR
                    for q in range(q0, q1 + 1):
                        lo, hi = max(moff, NR * q), min(moff + mp, NR * (q + 1))
                        nc.sync.dma_start(
                            out=a2a_in[4 * (g // 2) + q, lo - NR * q:hi - NR * q,
                                       g % 2, :],
                            in_=xn[lo - moff:hi - moff, :])
                    if debug:
                        nc.sync.dma_start(out=dbg["a"][g, moff:moff + mp, :],
                                          in_=xn[:mp, :])

        mark("a2a")
        # =====================================================================
        # AllToAll reshard
        # =====================================================================
        nc.gpsimd.collective_compute("AllToAll", ALU.bypass, replica_groups=groups,
                                     ins=[a2a_in[:]], outs=[a2a_out[:]])

        def load_x_window(dst_ap, wg, first):
            if first:
                base = a2a_out[:]
                src = bass.AP(tensor=base.tensor,
                              offset=base.offset + 8 * wg * 2 * C,
                              ap=[[2 * C, 8], [C, 2], [NR * 2 * C, 8], [1, C]])
            else:
                src = x_dram[128 * wg:128 * (wg + 1), :]
            nc.sync.dma_start(out=dst_ap, in_=src)

        # =====================================================================
        # Temporal attention (t1 / t2)
        # =====================================================================
        def temporal(prefix, dbg_key, first):
            """Temporal attention, redesigned:
            - rel-pos scores P = q_true . rk computed token-major per window,
              sheared into [tok, 16] layout via a single linear-AP DMA load
              from a DRAM bounce (per head per half).
            - block-diag mask folded in as a -3e4 bias preloaded into PSUM
              together with the rel-score broadcast; QK matmuls accumulate
              on top, so exp directly yields masked scores and per-window Z.
            - o2 (rel-v) accumulated into the AV PSUM via 16 pre-tiled rvB
              matmuls on the transposed A; no aD/aDT/strided adds.
            """
            with ExitStack() as ph:
                wp = ph.enter_context(tc.tile_pool(name="wpT", bufs=1))
                zp = ph.enter_context(tc.tile_pool(name="zpT", bufs=1))
                qp = ph.enter_context(tc.tile_pool(name="qpT", bufs=1))
                ap_ = ph.enter_context(tc.tile_pool(name="apT", bufs=2))
                op_ = ph.enter_context(tc.tile_pool(name="opT", bufs=1))
                psp = ph.enter_context(tc.tile_pool(name="psT", bufs=2,
                                                    space="PSUM"))

                wq = load_w_cin(wp, f"{prefix}_wq", C)
                wk = load_w_cin(wp, f"{prefix}_wk", C)
                wv = load_w_cin(wp, f"{prefix}_wv", C)
                wo = load_wo(wp, f"{prefix}_wo")
                bq = load_bias_h(wp, f"{prefix}_bq")
                bo_b = bcast_tile(wp, f"{prefix}_bo")
                rkT = wp.tile([DH, NREL], BF16, tag="rkT")
                nc.sync.dma_start(out=rkT[:], in_=wts[f"{prefix}_rkT"][:])
                rvB = wp.tile([128, T, DH], BF16, tag="rvB")
                nc.sync.dma_start(out=rvB[:], in_=wts[f"{prefix}_rvB"][:])

                def proj_fm_t(zT, w_sb, out_tile, bias=None):
                    """[80, HEADS, 1152]; 128-wide tails merged 4 heads/bank."""
                    ptt = [psp.tile([128, 512], F32, tag="pst", bufs=2,
                                    name=f"ptt{_i}")
                           for _i in range(2)]
                    for h in range(HEADS):
                        for (o, n) in ((0, 512), (512, 512)):
                            pt = psp.tile([128, 512], F32, tag="ps")
                            for ci in range(CHUNKS):
                                nc.tensor.matmul(pt[:DH, :n],
                                                 w_sb[:, ci, DH * h:DH * (h + 1)],
                                                 zT[:, ci, o:o + n],
                                                 start=(ci == 0),
                                                 stop=(ci == CHUNKS - 1))
                            if bias is not None:
                                nc.vector.tensor_scalar_add(
                                    out=out_tile[:, h, o:o + n], in0=pt[:DH, :n],
                                    scalar1=bias[:, h:h + 1])
                            else:
                                rr_copy(out_tile[:, h, o:o + n], pt[:DH, :n])
                        for ci in range(CHUNKS):
                            nc.tensor.matmul(
                                ptt[h // 4][:DH, 128 * (h % 4):128 * (h % 4) + 128],
                                w_sb[:, ci, DH * h:DH * (h + 1)],
                                zT[:, ci, 1024:1152],
                                start=(ci == 0), stop=(ci == CHUNKS - 1))
                    for half4 in range(2):
                        tview = bass.AP(
                            tensor=out_tile.tensor,
                            offset=out_tile[:, :, :].offset
                            + 4 * half4 * (HALFW * 128) + 1024,
                            ap=[list(out_tile[:, :, :].ap[0]),
                                [HALFW * 128, 4], [1, 128]])
                        if bias is not None:
                            bv_ = bass.AP(tensor=bias.tensor,
                                          offset=bias[:, :].offset + 4 * half4,
                                          ap=[list(bias[:, :].ap[0]), [1, 4],
                                              [0, 128]])
                            nc.vector.tensor_tensor(out=tview,
                                                    in0=ptt[half4][0:DH, :],
                                                    in1=bv_, op=ALU.add)
                        else:
                            nc.vector.tensor_scalar_mul(
                                out=tview, in0=ptt[half4][0:DH, :], scalar1=1.0)

                QUADS = [(0, 4), (4, 4), (8, 1)]
                for half in range(2):
                    wlo = half * HALFW
                    ntok = 128 * HALFW  # 1152
                    xw = zp.tile([128, HALFW, C], F32, tag="xw")
                    for w in range(HALFW):
                        load_x_window(xw[:, w, :], wlo + w, first)
                    zT = zp.tile([128, CHUNKS, ntok], BF16, tag="zTt")
                    ln_to_fm(psp, xw, zT, HALFW)

                    qT = qp.tile([DH, HEADS, ntok], BF16, tag="qt")
                    kT = qp.tile([DH, HEADS, ntok], BF16, tag="kt")
                    proj_fm_t(zT, wq, qT, bias=bq)
                    proj_fm_t(zT, wk, kT, bias=None)
                    v = qp.tile([128, HALFW, C], BF16, tag="vt")
                    for w in range(HALFW):
                        pt = psp.tile([128, 512], F32, tag="ps")
                        pt2 = psp.tile([128, 512], F32, tag="ps")
                        for ci in range(CHUNKS):
                            nc.tensor.matmul(pt[:, :],
                                             zT[:, ci, 128 * w:128 * (w + 1)],
                                             wv[:, ci, 0:512],
                                             start=(ci == 0),
                                             stop=(ci == CHUNKS - 1))
                            nc.tensor.matmul(pt2[:, 0:128],
                                             zT[:, ci, 128 * w:128 * (w + 1)],
                                             wv[:, ci, 512:640],
                                             start=(ci == 0),
                                             stop=(ci == CHUNKS - 1))
                        rr_copy(v[:, w, 0:512], pt[:, :])
                        rr_copy(v[:, w, 512:640], pt2[:, 0:128])

                    # P = q_true . rk, token-major, bounced via DRAM for the
                    # per-partition shear (linear AP on the reload)
                    oT = op_.tile([DH, HEADS, ntok], BF16, tag="ot")
                    for h in range(HEADS):
                        pP = psp.tile([128, 512], F32, tag="pst", bufs=2)
                        for w in range(HALFW):
                            nc.tensor.matmul(pP[:, 33 * w:33 * w + 33],
                                             qT[:, h, 128 * w:128 * (w + 1)],
                                             rkT[:, :], start=True, stop=True)
                        # pSB2[p, j, w] = P[(w,p), j]; DRAM linear
                        # p*297 + 9j + w so the shear-load is 3-dim:
                        # (rl,t,t',w) -> [[4752,8],[288,16],[1,144]]+144
                        pSB2 = ap_.tile([128, NREL, HALFW], BF16, tag="pSB")
                        dstv = bass.AP(tensor=pSB2.tensor,
                                       offset=pSB2[:, :, :].offset,
                                       ap=[list(pSB2[:, :, :].ap[0]),
                                           [1, HALFW], [HALFW, NREL]])
                        rr_copy(dstv, pP[:, 0:HALFW * NREL])
                        nc.sync.dma_start(out=p_dram[h, half],
                                          in_=pSB2[:].rearrange(
                                              "p a b -> p (a b)"))
                        s2h = ap_.tile([128, 16, HALFW], BF16, tag="s2h")
                        base = p_dram[h, half]
                        shear = bass.AP(
                            tensor=base.tensor, offset=base.offset + 16 * HALFW,
                            ap=[[16 * NREL * HALFW, 8],
                                [(NREL - 1) * HALFW, 16], [1, 16 * HALFW]])
                        nc.sync.dma_start(
                            out=s2h[:].rearrange("p a b -> p (a b)"), in_=shear)
                        if debug and dbg_key == "t1" and half == 0:
                            nc.gpsimd.dma_start(out=dbg["s2"][h], in_=s2h[:])

                        e2 = ap_.tile([128, 16 * HALFW], BF16, tag="e2")
                        nc.scalar.activation(
                            out=e2[:], in_=s2h[:].rearrange("p a b -> p (a b)"),
                            func=AF.Exp, scale=SCALE)
                        for (q0, nw) in QUADS:
                            pre = psp.tile([128, 512], F32, tag="pre", bufs=2)
                            for i in range(nw):
                                w = q0 + i
                                nc.tensor.matmul(
                                    pre[:, 128 * i:128 * (i + 1)],
                                    qT[:, h, 128 * w:128 * (w + 1)],
                                    kT[:, h, 128 * w:128 * (w + 1)],
                                    start=True, stop=True)
                            aU = ap_.tile([128, 4, 128], BF16, tag="aU")
                            zs = small.tile([128, 4], F32, tag="zsT")
                            for i in range(nw):
                                w = q0 + i
                                e2m = ap_.tile([128, 128], BF16, tag="e2m")
                                e2view = bass.AP(
                                    tensor=e2.tensor,
                                    offset=e2[:, :].offset + w,
                                    ap=[list(e2[:, :].ap[0]), [0, 8],
                                        [HALFW, 16]])
                                nc.gpsimd.tensor_tensor(
                                    out=e2m[:], in0=e2view, in1=maskb[:, :],
                                    op=ALU.mult)
                                exw = ap_.tile([128, 128], BF16, tag="exw")
                                nc.scalar.activation(
                                    out=exw[:],
                                    in_=pre[:, 128 * i:128 * (i + 1)],
                                    func=AF.Exp, scale=SCALE)
                                nc.vector.scalar_tensor_tensor(
                                    out=aU[:, i, :], in0=exw[:], scalar=1.0,
                                    in1=e2m[:], op0=ALU.mult, op1=ALU.mult,
                                    accum_out=zs[:, i:i + 1])
                            nc.vector.reciprocal(out=zs[:, 0:nw],
                                                 in_=zs[:, 0:nw])
                            for i in range(nw):
                                nc.vector.tensor_scalar_mul(
                                    out=aU[:, i, :], in0=aU[:, i, :],
                                    scalar1=zs[:, i:i + 1])
                            if debug and dbg_key == "t1" and half == 0 \
                                    and h == 0:
                                for i in range(nw):
                                    nc.gpsimd.dma_start(
                                        out=dbg["aU"][q0 + i],
                                        in_=aU[:, i, :])
                            for i in range(nw):
                                w = q0 + i
                                paT = psp.tile([128, 128], BF16, tag="ps")
                                nc.tensor.transpose(paT[:], aU[:, i, :],
                                                    identb[:])
                                aTs = ap_.tile([128, 128], BF16, tag="aTs")
                                rr_copy(aTs[:, :], paT[:, :])
                                po = psp.tile([128, 128], F32, tag="po",
                                              bufs=2)
                                nc.tensor.matmul(po[0:DH, :],
                                                 v[:, w, DH * h:DH * (h + 1)],
                                                 aTs[:, :],
                                                 start=True, stop=False)
                                for t in range(T):
                                    rhs = bass.AP(
                                        tensor=aTs.tensor,
                                        offset=aTs[:, :].offset + t,
                                        ap=[list(aTs[:, :].ap[0]), [16, 8]])
                                    dst = bass.AP(
                                        tensor=po.tensor,
                                        offset=po[:, :].offset + t,
                                        ap=[list(po[0:DH, :].ap[0]), [16, 8]])
                                    nc.tensor.matmul(dst, rvB[:, t, :], rhs,
                                                     start=False,
                                                     stop=(t == T - 1),
                                                     skip_group_check=True)
                                rr_copy(oT[:, h, 128 * w:128 * (w + 1)],
                                        po[0:DH, :])

                    if debug and dbg_key == "t1" and half == 0:
                        for nm, t_ in (("qT", qT), ("kT", kT), ("oT", oT)):
                            nc.gpsimd.dma_start(out=dbg[nm][:], in_=t_[:])
                        nc.gpsimd.dma_start(out=dbg["vt"][:], in_=v[:])
                    for w in range(HALFW):
                        wg = wlo + w
                        wo_residual(psp, oT, wo, w, xw[:, w, :], bo_b)
                        nc.sync.dma_start(out=x_dram[128 * wg:128 * (wg + 1), :],
                                          in_=xw[:, w, :])
                        if debug:
                            nc.sync.dma_start(
                                out=dbg[dbg_key][:].rearrange(
                                    "r t c -> (r t) c")[128 * wg:128 * (wg + 1), :],
                                in_=xw[:, w, :])

        mark("t1")
        temporal("t1", "t1", first=True)

        mark("X")
        # =====================================================================
        # Cross-attention (processed in halves)
        # =====================================================================
        with ExitStack() as ph:
            wp = ph.enter_context(tc.tile_pool(name="wpX", bufs=1))
            zp = ph.enter_context(tc.tile_pool(name="zpX", bufs=2))
            qp = ph.enter_context(tc.tile_pool(name="qpX", bufs=1))
            ap_ = ph.enter_context(tc.tile_pool(name="apX", bufs=2))
            op_ = ph.enter_context(tc.tile_pool(name="opX", bufs=2))
            psp = ph.enter_context(tc.tile_pool(name="psX", bufs=8, space="PSUM"))

            wq = load_w_cin(wp, "a2_wq", C)
            wkc = load_w_cin(wp, "a2_wk", CTXD)
            wvc = load_w_cin(wp, "a2_wv", CTXD)
            wo = load_wo(wp, "a2_wo")
            bq = load_bias_h(wp, "a2_bq")
            bo_b = bcast_tile(wp, "a2_bo")

            ctx_sb = wp.tile([128, CTXCH, 77], BF16, tag="ctx")
            nc.sync.dma_start(out=ctx_sb[:],
                              in_=ctxT_in[:].rearrange("(a p) m -> p a m", p=128))
            kctxT = wp.tile([DH, HEADS, 77], BF16, tag="kctx")
            for h in range(HEADS):
                pt = psp.tile([128, 77], F32, tag="ps")
                for ci in range(CTXCH):
                    nc.tensor.matmul(pt[:DH, :], wkc[:, ci, DH * h:DH * (h + 1)],
                                     ctx_sb[:, ci, :],
                                     start=(ci == 0), stop=(ci == CTXCH - 1))
                nc.scalar.copy(out=kctxT[:, h, :], in_=pt[:DH, :])
            vctx = wp.tile([77, C], BF16, tag="vctx")
            for (o, n) in nsplits(C):
                pt = psp.tile([77, 512], F32, tag="ps")
                for ci in range(CTXCH):
                    nc.tensor.matmul(pt[:, :n], ctx_sb[:, ci, :],
                                     wvc[:, ci, o:o + n],
                                     start=(ci == 0), stop=(ci == CTXCH - 1))
                nc.scalar.copy(out=vctx[:, o:o + n], in_=pt[:, :n])

            for half in range(2):
                wlo = half * HALFW
                ntok = 128 * HALFW
                xw = zp.tile([128, HALFW, C], F32, tag="xwx")
                for w in range(HALFW):
                    load_x_window(xw[:, w, :], wlo + w, False)
                zT = zp.tile([128, CHUNKS, ntok], BF16, tag="zTx")
                ln_to_fm(psp, xw, zT, HALFW)
                qT = qp.tile([DH, HEADS, ntok], BF16, tag="qx")
                proj_heads(psp, zT, wq, qT, ntok, bias=bq)

                oT = op_.tile([DH, HEADS, ntok], BF16, tag="ox")
                for w in range(HALFW):
                    for h in range(HEADS):
                        ps = psp.tile([128, 77], F32, tag="ps")
                        nc.tensor.matmul(ps[:], qT[:, h, 128 * w:128 * (w + 1)],
                                         kctxT[:, h, :], start=True, stop=True)
                        ex = ap_.tile([128, 77], F32, tag="exx")
                        zsum = small.tile([128, 1], F32, tag="zsX")
                        nc.scalar.activation(out=ex[:], in_=ps[:], func=AF.Exp,
                                             scale=SCALE, accum_out=zsum[:])
                        nc.vector.reciprocal(out=zsum[:], in_=zsum[:])
                        ab = ap_.tile([128, 77], BF16, tag="abx")
                        nc.vector.tensor_scalar_mul(out=ab[:], in0=ex[:],
                                                    scalar1=zsum[:])
                        paT = psp.tile([77, 128], BF16, tag="ps")
                        nc.tensor.transpose(paT[:], ab[:], identb[:])
                        aT = ap_.tile([77, 128], BF16, tag="aTx")
                        nc.scalar.copy(out=aT[:], in_=paT[:])
                        po = psp.tile([DH, 128], F32, tag="ps")
                        nc.tensor.matmul(po[:], vctx[:, DH * h:DH * (h + 1)],
                                         aT[:], start=True, stop=True)
                        nc.scalar.copy(out=oT[:, h, 128 * w:128 * (w + 1)],
                                       in_=po[:])
                for w in range(HALFW):
                    wg = wlo + w
                    wo_residual(psp, oT, wo, w, xw[:, w, :], bo_b)
                    nc.sync.dma_start(out=x_dram[128 * wg:128 * (wg + 1), :],
                                      in_=xw[:, w, :])
                    if debug:
                        nc.sync.dma_start(
                            out=dbg["x2"][:].rearrange(
                                "r t c -> (r t) c")[128 * wg:128 * (wg + 1), :],
                            in_=xw[:, w, :])

        mark("t2")
        temporal("t2", "t2", first=False)

        mark("FFN")
        # =====================================================================
        # GEGLU FFN (slices of 3 windows)
        # =====================================================================
        with ExitStack() as ph:
            wp = ph.enter_context(tc.tile_pool(name="wpF", bufs=1))
            zp = ph.enter_context(tc.tile_pool(name="zpF", bufs=1))
            hp = ph.enter_context(tc.tile_pool(name="hpF", bufs=2))
            psp = ph.enter_context(tc.tile_pool(name="psF", bufs=8, space="PSUM"))

            w1 = wp.tile([128, CHUNKS, 2 * FFI], BF16, tag="w1")
            nc.sync.dma_start(out=w1[:],
                              in_=wts["ff_w1"][:].rearrange("(a p) n -> p a n",
                                                            p=128))
            w2 = wp.tile([128, FFI // 128, C], BF16, tag="w2")
            nc.sync.dma_start(out=w2[:],
                              in_=wts["ff_w2"][:].rearrange("(a p) n -> p a n",
                                                            p=128))
            b1 = wp.tile([128, 2 * FFI // 128], F32, tag="b1")
            nc.sync.dma_start(out=b1[:],
                              in_=biases["ff_b1"][:].rearrange("(a p) -> p a",
                                                               p=128))
            fb2 = bcast_tile(wp, "ff_b2")

            SW = 3
            NG2 = FFI // 128  # 20
            for s in range(NWIN // SW):
                wlo = s * SW
                ntok = 128 * SW
                xw = zp.tile([128, SW, C], F32, tag="xwf")
                for w in range(SW):
                    load_x_window(xw[:, w, :], wlo + w, False)
                zT = zp.tile([128, CHUNKS, ntok], BF16, tag="zTf")
                ln_to_fm(psp, xw, zT, SW)
                hT = hp.tile([128, 2 * NG2, ntok], BF16, tag="hT")
                for co in range(2 * NG2):
                    pt = psp.tile([128, ntok], F32, tag="ps")
                    for ci in range(CHUNKS):
                        nc.tensor.matmul(pt[:], w1[:, ci, 128 * co:128 * (co + 1)],
                                         zT[:, ci, :],
                                         start=(ci == 0), stop=(ci == CHUNKS - 1))
                    nc.scalar.activation(out=hT[:, co, :], in_=pt[:],
                                         func=AF.Identity,
                                         bias=b1[:, co:co + 1], scale=1.0)
                uT = hp.tile([128, NG2, ntok], BF16, tag="uT")
                for co in range(NG2):
                    gl = hp.tile([128, ntok], BF16, tag="gelu")
                    nc.scalar.activation(out=gl[:], in_=hT[:, NG2 + co, :],
                                         func=AF.Gelu)
                    nc.vector.tensor_mul(out=uT[:, co, :], in0=hT[:, co, :],
                                         in1=gl[:])
                for w in range(SW):
                    wg = wlo + w
                    for (o, n) in nsplits(C):
                        pw = psp.tile([128, 512], F32, tag="ps")
                        for ci in range(NG2):
                            nc.tensor.matmul(pw[:, :n],
                                             uT[:, ci, 128 * w:128 * (w + 1)],
                                             w2[:, ci, o:o + n],
                                             start=(ci == 0), stop=(ci == NG2 - 1))
                        nc.vector.tensor_add(out=xw[:, w, o:o + n], in0=pw[:, :n],
                                             in1=xw[:, w, o:o + n])
                    nc.vector.tensor_add(out=xw[:, w, :], in0=xw[:, w, :],
                                         in1=fb2[:])
                    nc.sync.dma_start(
                        out=out_final[:].rearrange(
                            "r t c -> (r t) c")[128 * wg:128 * (wg + 1), :],
                        in_=xw[:, w, :])

    if not nc.is_finalized():
        nc.finalize()
    return nc


# ----------------------------------------------------------------------------
# host side
# ----------------------------------------------------------------------------

def _bf(a):
    return np.asarray(a, dtype=ml_dtypes.bfloat16)


def prepare_inputs(inputs):
    f = {k: np.asarray(v, dtype=np.float32) for k, v in inputs.items()}
    shared = {}

    def fold(g, b, wname):
        wf = f[wname]
        return f[g][:, None] * wf, f[b] @ wf

    for p, gk, bk_ in (("a1", "g1", "b1"), ("t1", "g4", "b4"),
                       ("t2", "g5", "b5")):
        for kind in ("wq", "wk", "wv"):
            wf, bias = fold(gk, bk_, f"{p}_{kind}")
            shared[f"{p}_{kind}"] = _bf(wf)
            shared[f"{p}_b{kind[1]}"] = bias.astype(np.float32)

    wf, bias = fold("g2", "b2", "a2_wq")
    shared["a2_wq"] = _bf(wf)
    shared["a2_bq"] = bias.astype(np.float32)
    shared["a2_wk"] = _bf(f["a2_wk"])
    shared["a2_wv"] = _bf(f["a2_wv"])
    shared["a2_bk"] = np.zeros(INNER, np.float32)
    shared["a2_bv"] = np.zeros(INNER, np.float32)
    for p in ("a1", "a2", "t1", "t2"):
        shared[f"{p}_wo"] = _bf(
            f[f"{p}_wo"].reshape(HEADS, DH, C).transpose(1, 0, 2))
        shared[f"{p}_bo"] = f[f"{p}_bo"]
    # phase A S^T scheme: bk dropped (softmax-invariant), bv folded into bo
    shared["a1_bo"] = (f["a1_bo"].astype(np.float64)
                       + shared["a1_bv"].astype(np.float64)
                       @ f["a1_wo"].astype(np.float64)).astype(np.float32)
    for p in ("t1", "t2"):
        shared[f"{p}_rkT"] = _bf(f[f"{p}_rk"].T)
        rv = f[f"{p}_rv"]
        rvB = np.zeros((8, 16, T, DH), np.float32)
        for tp in range(16):
            for t in range(T):
                rvB[:, tp, t] = rv[tp - t + MAXREL]
        shared[f"{p}_rvB"] = _bf(rvB.reshape(128, T, DH))
        # bk dropped (softmax-invariant), bv folded into bo
        shared[f"{p}_bo"] = (f[f"{p}_bo"].astype(np.float64)
                             + shared[f"{p}_bv"].astype(np.float64)
                             @ f[f"{p}_wo"].astype(np.float64)).astype(np.float32)
    w1f, b1f = fold("g3", "b3", "ff_w1")
    shared["ff_w1"] = _bf(w1f)
    shared["ff_b1"] = (b1f + f["ff_b1"]).astype(np.float32)
    shared["ff_w2"] = _bf(f["ff_w2"])
    shared["ff_b2"] = f["ff_b2"]
    m = np.zeros((128, 128), np.float32)
    for g in range(8):
        m[16 * g:16 * (g + 1), 16 * g:16 * (g + 1)] = 1.0
    shared["bd_mask"] = _bf(m)

    x = f["x"]
    ctx = f["context"]
    in_maps = []
    for core in range(NCORES):
        im = dict(shared)
        xs = np.empty((NG, SEQ, C), np.float32)
        for g in range(NG):
            bt = core + 8 * g
            b, t = bt // T, bt % T
            xs[g] = x[b, :, t].reshape(C, SEQ).T
        im["xs_in"] = xs
        im["ctxT"] = _bf(ctx[core // 4].T.copy())
        in_maps.append(im)
    return in_maps


_PROGRAM_CACHE = {}


def run(inputs, debug=False, trace=False):
    key = "dbg" if debug else "plain"
    if key not in _PROGRAM_CACHE:
        _PROGRAM_CACHE[key] = build_program(debug=debug)
    nc = _PROGRAM_CACHE[key]
    in_maps = prepare_inputs(inputs)
    from concourse.bass_utils import run_bass_kernel_spmd
    res = run_bass_kernel_spmd(nc, in_maps, list(range(NCORES)), trace=trace)
    outs = res.results
    full = np.empty((B * H * W, T, C), np.float32)
    for core in range(NCORES):
        full[NR * core:NR * (core + 1)] = outs[core]["out"]
    y = full.reshape(B, H, W, T, C).transpose(0, 4, 3, 1, 2)
    return y, res, outs


def kernel(**inputs):
    y, _, _ = run(inputs)
    return y.astype(np.float32)

